# revision 2
# baseline (speedup 1.0000x reference)
"""GAT (graph attention) layer on 8 Trainium2 NeuronCores — v2.

Strategy (dst-partitioned edge parallelism, bulk SWDGE gathers):
  - Nodes split into 8 contiguous row-ranges (one per core). Per core,
    dst nodes are packed into NW windows of <=127 nodes.
  - Phase A (per core): project the core's node shard with TensorE:
        [Wh | e_s | e_d] = h_shard @ [Wmat | A_s | A_d] + bias
    Rows [Wh | e_s] (264 bf16, padded to 384 = 768B) go to a DRAM table
    that is AllGathered so every core holds all 50176 rows; e_d rows
    (8 f32, padded to 64 = 256B) stay in a core-local DRAM mini-table.
  - Phase B (per core): edges grouped by destination window; G windows
    form a "group". Per group, three bulk dma_gather calls fetch all
    per-edge data (dma_gather's int16 indices only address 32768 rows,
    so the global table is read through two base offsets — "lo" rows
    [0, 32768) and "hi" rows [32768, NSG) — and every 128-edge chunk is
    packed all-lo or all-hi on the host):
      * table rows by src (one call per lo/hi section)
      * e_d rows by dst from the local mini-table (single call; dst is
        always in the local shard)
    Then per group: w = max(exp(e_s+e_d), exp(0.2(e_s+e_d))) (ACT+DVE),
    one batched DVE op builds all 128-edge one-hot matrices, and per
    window C matmuls accumulate [sum w*Wh | sum w] in PSUM, normalized
    once per window (equivalent to the reference softmax; scores are
    bounded so the max-subtraction is unnecessary).
  - Host does only index/layout work: edge bucketing, packing,
    transposes, and final row/column unscrambles.
"""
import os
import sys

sys.path.insert(0, "/opt/trn_rl_repo")

import numpy as np
import ml_dtypes

import concourse.bass as bass
import concourse.bacc as bacc
import concourse.tile as tile
import concourse.mybir as mybir

BF16 = np.dtype(ml_dtypes.bfloat16)
P = 128
SPLIT = 32768          # dma_gather int16 index limit

FULL_CFG = dict(
    N=50000, F=512, H=8, O=32, ALPHA=0.2, NCORES=8,
    NS=6272, NW=50, G=2,
)

_LAST_RESULTS = {}  # exposed for test.py (exec time etc.)


# --------------------------------------------------------------------------
# Host-side planning
# --------------------------------------------------------------------------

def _plan(cfg, src, dst):
    """Window assignment + lo/hi chunk packing, equalized across cores.

    Returns per-core index arrays (shared shapes) + group metadata
    (identical across cores, baked into the SPMD program).
    """
    import heapq

    N, NCORES = cfg["N"], cfg["NCORES"]
    NS, NW, G = cfg["NS"], cfg["NW"], cfg["G"]
    NG = NW // G
    assert NW % G == 0

    deg = np.bincount(dst, minlength=N).astype(np.int64)

    slot_of = np.empty(N, np.int32)   # window within core
    pos_of = np.empty(N, np.int32)    # position within window (0..126)

    for c in range(NCORES):
        lo, hi = NS * c, min(NS * (c + 1), N)
        nodes = np.arange(lo, hi)
        order = nodes[np.argsort(-deg[lo:hi], kind="stable")]
        heap = [(0, 0, w) for w in range(NW)]
        heapq.heapify(heap)
        for n in order:
            load, cnt, w = heapq.heappop(heap)
            slot_of[n] = w
            pos_of[n] = cnt
            cnt += 1
            load += deg[n]
            if cnt < P - 1:  # positions 0..126; 127 reserved for pads
                heapq.heappush(heap, (load, cnt, w))

    # per (core, window, half) edge lists
    core_of = dst // NS
    ed_lists = [[[None, None] for _ in range(NW)] for _ in range(NCORES)]
    for c in range(NCORES):
        m = core_of == c
        s_c, d_c = src[m], dst[m]
        w_c = slot_of[d_c]
        half = (s_c >= SPLIT).astype(np.int64)
        key = w_c * 2 + half
        order = np.argsort(key, kind="stable")
        s_c, d_c, key = s_c[order], d_c[order], key[order]
        bounds = np.searchsorted(key, np.arange(2 * NW + 1))
        for w in range(NW):
            for h in (0, 1):
                a, b = bounds[2 * w + h], bounds[2 * w + h + 1]
                ed_lists[c][w][h] = (s_c[a:b], d_c[a:b])

    # chunk counts per (window, half), equalized across cores
    n_chunks = np.zeros((NW, 2), np.int64)
    for c in range(NCORES):
        for w in range(NW):
            for h in (0, 1):
                n = len(ed_lists[c][w][h][0])
                n_chunks[w, h] = max(n_chunks[w, h], -(-n // P))

    # group metadata (identical across cores)
    groups = []
    lo8_off = hi8_off = ed8_off = col_off = 0
    for g in range(NG):
        wins = list(range(g * G, (g + 1) * G))
        K_lo = int(sum(n_chunks[w, 0] for w in wins))
        K_hi = int(sum(n_chunks[w, 1] for w in wins))
        K = K_lo + K_hi
        win_meta = []
        lo_rel = hi_rel = 0
        for w in wins:
            nl, nh = int(n_chunks[w, 0]), int(n_chunks[w, 1])
            win_meta.append(dict(w=w, lo_rel=lo_rel, n_lo=nl,
                                 hi_rel=hi_rel, n_hi=nh))
            lo_rel += nl
            hi_rel += nh
        groups.append(dict(K_lo=K_lo, K_hi=K_hi, K=K,
                           lo8_off=lo8_off, hi8_off=hi8_off,
                           ed8_off=ed8_off, col_off=col_off,
                           wins=win_meta))
        lo8_off += K_lo * 8
        hi8_off += K_hi * 8
        ed8_off += K * 8
        col_off += K
    LO8, HI8, ED8, KT = lo8_off, hi8_off, ed8_off, col_off

    # per-core packed arrays
    idx_lo = np.zeros((NCORES, 128, LO8), np.int16)
    idx_hi = np.zeros((NCORES, 128, HI8), np.int16)
    idx_ed = np.zeros((NCORES, 128, ED8), np.int16)
    dstl = np.full((NCORES, P, KT), 127.0, np.float32)

    for c in range(NCORES):
        ilo = np.zeros(LO8 * 16, np.int64)   # flat idx pos -> table row
        ihi = np.zeros(HI8 * 16, np.int64)
        ied = np.zeros(ED8 * 16, np.int64)
        for gm in groups:
            for h, (ixarr, base8) in enumerate(
                    ((ilo, gm["lo8_off"]), (ihi, gm["hi8_off"]))):
                sec_rel = 0
                for wm in gm["wins"]:
                    s_e, d_e = ed_lists[c][wm["w"]][h]
                    n = len(s_e)
                    i = np.arange(n)
                    pos = base8 * 16 + (sec_rel + i // P) * P + i % P
                    ixarr[pos] = s_e - (SPLIT if h else 0)
                    # ed idx + dstl live in the unified col space
                    colb = gm["col_off"] + (gm["K_lo"] if h else 0)
                    col = colb + (wm["lo_rel"] if h == 0 else wm["hi_rel"]) \
                        + i // P
                    epos = gm["ed8_off"] * 16 \
                        + ((gm["K_lo"] if h else 0)
                           + (wm["lo_rel"] if h == 0 else wm["hi_rel"])
                           + i // P) * P + i % P
                    ied[epos] = d_e - NS * c
                    dstl[c][i % P, col] = pos_of[d_e]
                    sec_rel += (wm["n_lo"] if h == 0 else wm["n_hi"])
        for flat, arr16 in ((ilo, idx_lo), (ihi, idx_hi), (ied, idx_ed)):
            if flat.size == 0:
                continue
            a = np.zeros((16, flat.size // 16), np.int16)
            a[np.arange(flat.size) % 16, np.arange(flat.size) // 16] = flat
            arr16[c] = np.tile(a, (8, 1))

    return dict(
        groups=groups, LO8=LO8, HI8=HI8, ED8=ED8, KT=KT,
        idx_lo=idx_lo, idx_hi=idx_hi, idx_ed=idx_ed, dstl=dstl,
        slot_of=slot_of, pos_of=pos_of,
    )


def _host_weights(cfg, W, Wb, a, ab):
    """Extended projection weights / bias (o-major, h-inner layout)."""
    H, F, O = W.shape
    a_src, a_dst = a[:, :O], a[:, O:]
    Wmat = W.transpose(1, 2, 0).reshape(F, O * H)          # [F, (o,h)]
    A_s = np.einsum("hfo,ho->fh", W, a_src)
    A_d = np.einsum("hfo,ho->fh", W, a_dst)
    Wext = np.concatenate([Wmat, A_s, A_d], axis=1)        # [F, OH+2H]
    c_s = (Wb * a_src).sum(1)
    c_d = (Wb * a_dst).sum(1)
    bext = np.concatenate([Wb.T.reshape(-1), np.zeros(H, np.float32),
                           c_s + c_d + ab]).astype(np.float32)
    return Wext.astype(np.float32), bext


# --------------------------------------------------------------------------
# Device program
# --------------------------------------------------------------------------

def build_gat_bass(cfg, plan_meta, repeat=1):
    N, F, H, O, NCORES = cfg["N"], cfg["F"], cfg["H"], cfg["O"], cfg["NCORES"]
    NS, NW, G = cfg["NS"], cfg["NW"], cfg["G"]
    HO = H * O
    TDC = HO + H         # useful table row: Wh + e_s = 264
    ROWB = 384           # padded table row elems (768 B)
    EDW = 64             # padded e_d row elems f32 (256 B)
    AD = HO + 2 * H      # phase-A psum width = 272
    NT = NS // P
    KT_F = F // P
    NSG = NS * NCORES

    groups = plan_meta["groups"]
    LO8, HI8, ED8, KT = (plan_meta[k] for k in ("LO8", "HI8", "ED8", "KT"))

    bf = mybir.dt.bfloat16
    f32 = mybir.dt.float32
    i16 = mybir.dt.int16

    nc = bacc.Bacc("TRN2", target_bir_lowering=False, debug=False,
                   num_devices=NCORES, num_swdge_queues=4)

    hT = nc.dram_tensor("hT", [F, NS], bf, kind="ExternalInput")
    wext = nc.dram_tensor("wext", [F, AD], bf, kind="ExternalInput")
    bext = nc.dram_tensor("bext", [1, AD], bf, kind="ExternalInput")
    ones1 = nc.dram_tensor("ones1", [1, P], bf, kind="ExternalInput")
    iota = nc.dram_tensor("iota", [P, P], bf, kind="ExternalInput")
    idx_lo = nc.dram_tensor("idx_lo", [128, LO8], i16, kind="ExternalInput")
    idx_hi = nc.dram_tensor("idx_hi", [128, HI8], i16, kind="ExternalInput")
    idx_ed = nc.dram_tensor("idx_ed", [128, ED8], i16, kind="ExternalInput")
    dstl = nc.dram_tensor("dstl", [P, KT], bf, kind="ExternalInput")

    out_local = nc.dram_tensor("out_local", [NW * P, HO], f32,
                               kind="ExternalOutput")

    with tile.TileContext(nc) as tc:
      for _rep in range(repeat):
        with (
            tc.tile_pool(name="dram", bufs=1, space="DRAM") as dram,
            tc.tile_pool(name="const", bufs=1) as cpool,
        ):
            tbl_local = dram.tile([NS, ROWB], bf)
            tbl_global = dram.tile(
                [NSG, ROWB], bf,
                addr_space="Shared" if NCORES > 4 else "Local")
            ed_local = dram.tile([NS, EDW], f32)

            iota_t = cpool.tile([P, P], bf)
            nc.sync.dma_start(out=iota_t[:], in_=iota[:, :])
            ones_t = cpool.tile([1, P], bf)
            nc.sync.dma_start(out=ones_t[:], in_=ones1[:, :])
            bext_t = cpool.tile([1, AD], bf)
            nc.sync.dma_start(out=bext_t[:], in_=bext[:, :])
            ilo_t = cpool.tile([128, LO8], i16)
            nc.sync.dma_start(out=ilo_t[:], in_=idx_lo[:, :])
            ihi_t = cpool.tile([128, HI8], i16)
            nc.sync.dma_start(out=ihi_t[:], in_=idx_hi[:, :])
            ied_t = cpool.tile([128, ED8], i16)
            nc.sync.dma_start(out=ied_t[:], in_=idx_ed[:, :])
            dstl_t = cpool.tile([P, KT], bf)
            nc.sync.dma_start(out=dstl_t[:], in_=dstl[:, :])

            # ---------------- Phase A: projection ----------------
            with (
                tc.tile_pool(name="pa_sb", bufs=1) as pa,
                tc.tile_pool(name="pa_ps", bufs=2, space="PSUM") as pa_ps,
            ):
                hT_t = pa.tile([P, KT_F * NS], bf, tag="hT")
                for kk in range(KT_F):
                    nc.sync.dma_start(out=hT_t[:, kk * NS:(kk + 1) * NS],
                                      in_=hT[kk * P:(kk + 1) * P, :])
                wext_t = pa.tile([P, KT_F * AD], bf, tag="wext")
                for kk in range(KT_F):
                    nc.sync.dma_start(out=wext_t[:, kk * AD:(kk + 1) * AD],
                                      in_=wext[kk * P:(kk + 1) * P, :])

                stage = pa.tile([P, NT * ROWB], bf, tag="stage")
                nc.vector.memset(stage[:], 0.0)
                ed_stage = pa.tile([P, NT * EDW], f32, tag="ed_stage")
                nc.vector.memset(ed_stage[:], 0.0)

                for t in range(NT):
                    psA = pa_ps.tile([P, AD], f32, tag="psA")
                    for kk in range(KT_F):
                        nc.tensor.matmul(
                            out=psA[:],
                            lhsT=hT_t[:, kk * NS + t * P: kk * NS + (t + 1) * P],
                            rhs=wext_t[:, kk * AD:(kk + 1) * AD],
                            start=(kk == 0), stop=False)
                    nc.tensor.matmul(out=psA[:], lhsT=ones_t[:],
                                     rhs=bext_t[:], start=False, stop=True)
                    nc.vector.tensor_copy(
                        out=stage[:, t * ROWB:t * ROWB + TDC],
                        in_=psA[:, 0:TDC])
                    nc.vector.tensor_copy(
                        out=ed_stage[:, t * EDW:t * EDW + H],
                        in_=psA[:, TDC:TDC + H])

                nc.sync.dma_start(
                    out=tbl_local[:].rearrange("(t p) d -> p t d", p=P),
                    in_=stage[:].rearrange("p (t d) -> p t d", t=NT))
                nc.sync.dma_start(
                    out=ed_local[:].rearrange("(t p) d -> p t d", p=P),
                    in_=ed_stage[:].rearrange("p (t d) -> p t d", t=NT))

            if cfg.get("skip_collective"):
                nc.sync.dma_start(out=tbl_global[0:NS, :], in_=tbl_local[:])
            else:
                nc.gpsimd.collective_compute(
                    "AllGather",
                    mybir.AluOpType.bypass,
                    replica_groups=[list(range(NCORES))],
                    ins=[tbl_local.opt()],
                    outs=[tbl_global.opt()],
                )

            # ---------------- Phase B: edges ----------------
            with (
                tc.tile_pool(name="g_sb", bufs=2) as gp,
                tc.tile_pool(name="ed_sb", bufs=2) as edp,
                tc.tile_pool(name="w_sb", bufs=2) as wpool,
                tc.tile_pool(name="a_sb", bufs=2) as apool,
                tc.tile_pool(name="o_sb", bufs=2) as opool,
                tc.tile_pool(name="agg_ps", bufs=4, space="PSUM") as aggp,
            ):
                # HW limit: a dma_gather's descriptors must fit the SWDGE
                # ring -> cap num_idxs per call (empirically 512 ok, 1920
                # wedges the device). Rotate the 4 SWDGE queues so the next
                # call's descriptor generation overlaps in-flight transfers.
                CAP = int(os.environ.get("GAT_CAP", "7"))
                _q = [0]

                def capped_gather(dst_tile, col0, ncols, in_ap, idx_t,
                                  idx8_off, elem):
                    c, off, left = col0, idx8_off, ncols
                    while left > 0:
                        n = min(left, CAP)
                        nc.gpsimd.dma_gather(
                            dst_tile[:, c * elem:(c + n) * elem].rearrange(
                                "p (k d) -> p k d", k=n),
                            in_ap, idx_t[:, off:off + n * 8],
                            n * P, n * P, elem,
                            queue_num=_q[0])
                        _q[0] = (_q[0] + 1) % 4
                        c += n
                        off += n * 8
                        left -= n

                for gm in groups:
                    K, KL, KH = gm["K"], gm["K_lo"], gm["K_hi"]
                    g_t = gp.tile([P, K * ROWB], bf, tag="g")
                    if KL:
                        capped_gather(g_t, 0, KL, tbl_global[0:SPLIT, :],
                                      ilo_t, gm["lo8_off"], ROWB)
                    if KH:
                        capped_gather(g_t, KL, KH, tbl_global[SPLIT:NSG, :],
                                      ihi_t, gm["hi8_off"], ROWB)
                    ed_t = edp.tile([P, K * EDW], f32, tag="ed")
                    capped_gather(ed_t, 0, K, ed_local[:, :],
                                  ied_t, gm["ed8_off"], EDW)

                    g4 = g_t[:].rearrange("p (k d) -> p k d", k=K)
                    ed4 = ed_t[:].rearrange("p (k d) -> p k d", k=K)

                    # s = e_s + e_d
                    s_t = wpool.tile([P, K * H], f32, tag="s")
                    nc.vector.tensor_tensor(
                        out=s_t[:].rearrange("p (k x) -> p k x", k=K),
                        in0=g4[:, :, HO:TDC],
                        in1=ed4[:, :, 0:H],
                        op=mybir.AluOpType.add)

                    # w = max(exp(s), exp(0.2 s)) -> g cols HO:TDC (bf16)
                    w1 = wpool.tile([P, K * H], f32, tag="w1")
                    nc.scalar.activation(out=w1[:], in_=s_t[:],
                                         func=mybir.ActivationFunctionType.Exp)
                    w2 = wpool.tile([P, K * H], f32, tag="w2")
                    nc.scalar.activation(out=w2[:], in_=s_t[:],
                                         func=mybir.ActivationFunctionType.Exp,
                                         scale=float(cfg["ALPHA"]))
                    nc.vector.tensor_tensor(
                        out=g4[:, :, HO:TDC],
                        in0=w1[:].rearrange("p (k x) -> p k x", k=K),
                        in1=w2[:].rearrange("p (k x) -> p k x", k=K),
                        op=mybir.AluOpType.max)

                    # Wh *= w (broadcast over O), in place
                    nc.vector.tensor_tensor(
                        out=g4[:, :, 0:HO].rearrange(
                            "p k (o x) -> p k o x", o=O),
                        in0=g4[:, :, 0:HO].rearrange(
                            "p k (o x) -> p k o x", o=O),
                        in1=g4[:, :, HO:TDC][:, :, None, :].to_broadcast(
                            [P, K, O, H]),
                        op=mybir.AluOpType.mult)

                    # batched one-hot: a[p, k, v] = (iota[v] == dstl[p, k])
                    a_t = apool.tile([P, K * P], bf, tag="a")
                    nc.vector.tensor_tensor(
                        out=a_t[:].rearrange("p (k v) -> p k v", k=K),
                        in0=iota_t[:][:, None, :].to_broadcast([P, K, P]),
                        in1=dstl_t[:, gm["col_off"]:gm["col_off"] + K][
                            :, :, None].to_broadcast([P, K, P]),
                        op=mybir.AluOpType.is_equal)

                    for wm in gm["wins"]:
                        cols = (list(range(wm["lo_rel"],
                                           wm["lo_rel"] + wm["n_lo"]))
                                + list(range(KL + wm["hi_rel"],
                                             KL + wm["hi_rel"] + wm["n_hi"])))
                        o_t = opool.tile([P, HO], f32, tag="o")
                        if not cols:
                            nc.vector.memset(o_t[:], 0.0)
                        else:
                            agg = aggp.tile([P, TDC], f32, tag="agg")
                            for j, col in enumerate(cols):
                                nc.tensor.matmul(
                                    out=agg[:],
                                    lhsT=a_t[:, col * P:(col + 1) * P],
                                    rhs=g_t[:, col * ROWB:col * ROWB + TDC],
                                    start=(j == 0), stop=(j == len(cols) - 1))
                            den = opool.tile([P, H], f32, tag="den")
                            nc.vector.tensor_scalar(
                                out=den[:], in0=agg[:, HO:TDC],
                                scalar1=1e-30, scalar2=None,
                                op0=mybir.AluOpType.max)
                            rec = opool.tile([P, H], f32, tag="rec")
                            nc.vector.reciprocal(out=rec[:], in_=den[:])
                            nc.vector.tensor_tensor(
                                out=o_t[:].rearrange("p (o x) -> p o x", o=O),
                                in0=agg[:, 0:HO].rearrange(
                                    "p (o x) -> p o x", o=O),
                                in1=rec[:][:, None, :].to_broadcast([P, O, H]),
                                op=mybir.AluOpType.mult)
                        nc.sync.dma_start(
                            out=out_local[wm["w"] * P:(wm["w"] + 1) * P, :],
                            in_=o_t[:])

    return nc


# --------------------------------------------------------------------------
# Execution (PJRT via axon; chained-execution slope timing)
# --------------------------------------------------------------------------

def _run_pjrt_timed(nc, in_maps, n_cores, n_reps=1, chain=None):
    import time

    import jax
    from jax.sharding import Mesh, PartitionSpec
    from jax.experimental.shard_map import shard_map

    from concourse import bass2jax
    from concourse import mybir as mb

    bass2jax.install_neuronx_cc_hook()

    partition_name = (nc.partition_id_tensor.name
                      if nc.partition_id_tensor else None)

    in_names, out_names, out_avals, zero_outs = [], [], [], []
    for alloc in nc.m.functions[0].allocations:
        if not isinstance(alloc, mb.MemoryLocationSet):
            continue
        name = alloc.memorylocations[0].name
        if alloc.kind == "ExternalInput":
            if name != partition_name:
                in_names.append(name)
        elif alloc.kind == "ExternalOutput":
            shape = tuple(alloc.tensor_shape)
            dtype = mb.dt.np(alloc.dtype)
            out_names.append(name)
            out_avals.append(jax.core.ShapedArray(shape, dtype))
            zero_outs.append(np.zeros(shape, dtype))
    n_params = len(in_names)
    n_outs = len(out_avals)
    all_in_names = list(in_names) + out_names
    if partition_name is not None:
        all_in_names.append(partition_name)
    donate = tuple(range(n_params, n_params + n_outs))

    def _body(*args):
        operands = list(args)
        if partition_name is not None:
            operands.append(bass2jax.partition_id_tensor())
        outs = bass2jax._bass_exec_p.bind(
            *operands,
            out_avals=tuple(out_avals),
            in_names=tuple(all_in_names),
            out_names=tuple(out_names),
            lowering_input_output_aliases=(),
            sim_require_finite=True,
            sim_require_nnan=True,
            nc=nc,
        )
        return tuple(outs)

    devices = jax.devices()[:n_cores]
    mesh = Mesh(np.asarray(devices), ("core",))
    in_specs = (PartitionSpec("core"),) * (n_params + n_outs)
    out_specs = (PartitionSpec("core"),) * len(out_names)
    sharded = jax.jit(
        shard_map(_body, mesh=mesh, in_specs=in_specs, out_specs=out_specs,
                  check_rep=False),
        donate_argnums=donate, keep_unused=True)

    sharding = jax.sharding.NamedSharding(mesh, PartitionSpec("core"))
    concat_in = [
        jax.device_put(
            np.concatenate([np.asarray(in_maps[c][name])
                            for c in range(n_cores)], axis=0), sharding)
        for name in in_names
    ]

    def fresh_zeros():
        return [
            jax.device_put(
                np.zeros((n_cores * z.shape[0], *z.shape[1:]), z.dtype),
                sharding)
            for z in zero_outs
        ]

    def run_chain(k):
        outs = fresh_zeros()
        for o in outs:
            o.block_until_ready()
        t0 = time.perf_counter()
        for _ in range(k):
            outs = sharded(*concat_in, *outs)
        for o in outs:
            o.block_until_ready()
        return time.perf_counter() - t0, outs

    # warmup / compile
    _, out_arrs = run_chain(1)

    if chain:
        # Per-execution device time via chained-run slope. Chained calls
        # amortize the fixed dispatch latency; using the min totals at two
        # chain lengths rejects scheduler noise (fixed latency cancels).
        k1, k2 = chain
        t1s, t2s = [], []
        for _ in range(max(1, n_reps)):
            t1, _ = run_chain(k1)
            t2, out_arrs = run_chain(k2)
            t1s.append(t1)
            t2s.append(t2)
        slope = (min(t2s) - min(t1s)) / (k2 - k1)
        _LAST_RESULTS["wall_times_s"] = t1s + t2s
        _LAST_RESULTS["slopes_s"] = [
            (b - a) / (k2 - k1) for a, b in zip(t1s, t2s)]
        _LAST_RESULTS["exec_time_ns"] = int(slope * 1e9)
    else:
        times = []
        for _ in range(max(1, n_reps)):
            t, out_arrs = run_chain(1)
            times.append(t)
        _LAST_RESULTS["wall_times_s"] = times
        _LAST_RESULTS["exec_time_ns"] = int(min(times) * 1e9)

    return [
        {name: np.asarray(out_arrs[i]).reshape(n_cores, *out_avals[i].shape)[c]
         for i, name in enumerate(out_names)}
        for c in range(n_cores)
    ]


def _chain_totals(nc, in_maps, n_cores, k, n_reps):
    """Wall totals of `n_reps` chains of `k` dispatches of nc's program."""
    import time

    import jax
    from jax.sharding import Mesh, PartitionSpec
    from jax.experimental.shard_map import shard_map

    from concourse import bass2jax
    from concourse import mybir as mb

    bass2jax.install_neuronx_cc_hook()
    partition_name = (nc.partition_id_tensor.name
                      if nc.partition_id_tensor else None)
    in_names, out_names, out_avals, zero_outs = [], [], [], []
    for alloc in nc.m.functions[0].allocations:
        if not isinstance(alloc, mb.MemoryLocationSet):
            continue
        name = alloc.memorylocations[0].name
        if alloc.kind == "ExternalInput":
            if name != partition_name:
                in_names.append(name)
        elif alloc.kind == "ExternalOutput":
            shape = tuple(alloc.tensor_shape)
            dtype = mb.dt.np(alloc.dtype)
            out_names.append(name)
            out_avals.append(jax.core.ShapedArray(shape, dtype))
            zero_outs.append(np.zeros(shape, dtype))
    n_params = len(in_names)
    all_in_names = list(in_names) + out_names
    if partition_name is not None:
        all_in_names.append(partition_name)
    donate = tuple(range(n_params, n_params + len(out_avals)))

    def _body(*args):
        operands = list(args)
        if partition_name is not None:
            operands.append(bass2jax.partition_id_tensor())
        return tuple(bass2jax._bass_exec_p.bind(
            *operands, out_avals=tuple(out_avals),
            in_names=tuple(all_in_names), out_names=tuple(out_names),
            lowering_input_output_aliases=(),
            sim_require_finite=True, sim_require_nnan=True, nc=nc))

    devices = jax.devices()[:n_cores]
    mesh = Mesh(np.asarray(devices), ("core",))
    nio = n_params + len(out_avals)
    sharded = jax.jit(
        shard_map(_body, mesh=mesh, in_specs=(PartitionSpec("core"),) * nio,
                  out_specs=(PartitionSpec("core"),) * len(out_names),
                  check_rep=False),
        donate_argnums=donate, keep_unused=True)
    sharding = jax.sharding.NamedSharding(mesh, PartitionSpec("core"))
    concat_in = [
        jax.device_put(
            np.concatenate([np.asarray(in_maps[c][name])
                            for c in range(n_cores)], axis=0), sharding)
        for name in in_names
    ]

    def run_chain(kk):
        outs = [jax.device_put(
            np.zeros((n_cores * z.shape[0], *z.shape[1:]), z.dtype), sharding)
            for z in zero_outs]
        for o in outs:
            o.block_until_ready()
        t0 = time.perf_counter()
        for _ in range(kk):
            outs = sharded(*concat_in, *outs)
        for o in outs:
            o.block_until_ready()
        return time.perf_counter() - t0

    run_chain(1)  # warmup/compile
    return [run_chain(k) for _ in range(n_reps)]


# --------------------------------------------------------------------------
# Host entry point
# --------------------------------------------------------------------------

def _run(cfg, h, src, dst, W, Wb, a, ab, use_sim=False, timing=False):
    N, F, H, O, NCORES = cfg["N"], cfg["F"], cfg["H"], cfg["O"], cfg["NCORES"]
    NS, NW = cfg["NS"], cfg["NW"]
    HO = H * O

    h = np.asarray(h, np.float32)
    src = np.asarray(src).astype(np.int64)
    dst = np.asarray(dst).astype(np.int64)
    W = np.asarray(W, np.float32)
    Wb = np.asarray(Wb, np.float32)
    a = np.asarray(a, np.float32)
    ab = np.asarray(ab, np.float32)

    plan = _plan(cfg, src, dst)
    Wext, bext = _host_weights(cfg, W, Wb, a, ab)

    NSG = NS * NCORES
    h_pad = np.zeros((NSG, F), np.float32)
    h_pad[:N] = h
    iota_np = np.broadcast_to(np.arange(P, dtype=np.float32),
                              (P, P)).astype(BF16)
    in_maps = []
    for c in range(NCORES):
        in_maps.append({
            "hT": np.ascontiguousarray(
                h_pad[NS * c:NS * (c + 1)].T).astype(BF16),
            "wext": Wext.astype(BF16),
            "bext": bext.reshape(1, -1).astype(BF16),
            "ones1": np.ones((1, P), BF16),
            "iota": iota_np,
            "idx_lo": plan["idx_lo"][c],
            "idx_hi": plan["idx_hi"][c],
            "idx_ed": plan["idx_ed"][c],
            "dstl": plan["dstl"][c].astype(BF16),
        })

    nc = build_gat_bass(cfg, plan)
    nc.compile()

    if use_sim:
        from concourse import bass_interp
        sim = bass_interp.MultiCoreSim(nc, NCORES)
        for c in range(NCORES):
            for k, v in in_maps[c].items():
                sim.cores[c].tensor(k)[:] = v
        sim.simulate()
        outs = [np.array(sim.cores[c].mem_tensor("out_local"))
                for c in range(NCORES)]
    else:
        results = _run_pjrt_timed(nc, in_maps, NCORES, n_reps=1)
        outs = [results[c]["out_local"] for c in range(NCORES)]
        if timing:
            # Device-time measurement: the per-dispatch overhead of this
            # PJRT/axon path (~1.5 ms, measured with a trivial kernel) dwarfs
            # the kernel, so time a variant program whose body runs REP times
            # per dispatch and take the slope over REP via chained runs at
            # two chain lengths (min over reps cancels dispatch noise).
            REP, K, NR = cfg.get("REP", 5), 24, 3
            ncR = build_gat_bass(cfg, plan, repeat=REP)
            ncR.compile()
            t1s = _chain_totals(nc, in_maps, NCORES, K, NR)
            t2s = _chain_totals(ncR, in_maps, NCORES, K, NR)
            d = (min(t2s) - min(t1s)) / (K * (REP - 1))
            _LAST_RESULTS["wall_times_s"] = t1s + t2s
            _LAST_RESULTS["exec_time_ns"] = int(d * 1e9)

    # unscramble rows + columns
    slot_of, pos_of = plan["slot_of"], plan["pos_of"]
    nodes = np.arange(N)
    rows = slot_of[nodes] * P + pos_of[nodes]
    out = np.empty((N, HO), np.float32)
    for c in range(NCORES):
        lo, hi = NS * c, min(NS * (c + 1), N)
        out[lo:hi] = outs[c][rows[lo:hi]]
    # column map: ref col h*O+o <- ours o*H+h
    hh, oo = np.meshgrid(np.arange(H), np.arange(O), indexing="ij")
    colmap = (oo * H + hh).reshape(-1)
    return out[:, colmap]


def kernel(h, src, dst, W, Wb, a, ab):
    cfg = dict(FULL_CFG)
    timing = os.environ.get("GAT_TRACE", "0") == "1"
    use_sim = os.environ.get("GAT_SIM", "0") == "1"
    return _run(cfg, h, src, dst, W, Wb, a, ab, use_sim=use_sim,
                timing=timing)


# revision 3
# speedup vs baseline: 1.0828x; 1.0828x over previous
"""GAT (graph attention) layer on 8 Trainium2 NeuronCores — v2.

Strategy (dst-partitioned edge parallelism, bulk SWDGE gathers):
  - Nodes split into 8 contiguous row-ranges (one per core). Per core,
    dst nodes are packed into NW windows of <=127 nodes.
  - Phase A (per core): project the core's node shard with TensorE:
        [Wh | e_s | e_d] = h_shard @ [Wmat | A_s | A_d] + bias
    Rows [Wh | e_s] (264 bf16, padded to 384 = 768B) go to a DRAM table
    that is AllGathered so every core holds all 50176 rows; e_d rows
    (8 f32, padded to 64 = 256B) stay in a core-local DRAM mini-table.
  - Phase B (per core): edges grouped by destination window; G windows
    form a "group". Per group, three bulk dma_gather calls fetch all
    per-edge data (dma_gather's int16 indices only address 32768 rows,
    so the global table is read through two base offsets — "lo" rows
    [0, 32768) and "hi" rows [32768, NSG) — and every 128-edge chunk is
    packed all-lo or all-hi on the host):
      * table rows by src (one call per lo/hi section)
      * e_d rows by dst from the local mini-table (single call; dst is
        always in the local shard)
    Then per group: w = max(exp(e_s+e_d), exp(0.2(e_s+e_d))) (ACT+DVE),
    one batched DVE op builds all 128-edge one-hot matrices, and per
    window C matmuls accumulate [sum w*Wh | sum w] in PSUM, normalized
    once per window (equivalent to the reference softmax; scores are
    bounded so the max-subtraction is unnecessary).
  - Host does only index/layout work: edge bucketing, packing,
    transposes, and final row/column unscrambles.
"""
import os
import sys

sys.path.insert(0, "/opt/trn_rl_repo")

import numpy as np
import ml_dtypes

import concourse.bass as bass
import concourse.bacc as bacc
import concourse.tile as tile
import concourse.mybir as mybir

BF16 = np.dtype(ml_dtypes.bfloat16)
P = 128
SPLIT = 32768          # dma_gather int16 index limit

FULL_CFG = dict(
    N=50000, F=512, H=8, O=32, ALPHA=0.2, NCORES=8,
    NS=6272, NW=50, G=2,
)

_LAST_RESULTS = {}  # exposed for test.py (exec time etc.)


# --------------------------------------------------------------------------
# Host-side planning
# --------------------------------------------------------------------------

def _plan(cfg, src, dst):
    """Window assignment + lo/hi chunk packing, equalized across cores.

    Returns per-core index arrays (shared shapes) + group metadata
    (identical across cores, baked into the SPMD program).
    """
    import heapq

    N, NCORES = cfg["N"], cfg["NCORES"]
    NS, NW, G = cfg["NS"], cfg["NW"], cfg["G"]
    NG = NW // G
    assert NW % G == 0

    deg = np.bincount(dst, minlength=N).astype(np.int64)

    slot_of = np.empty(N, np.int32)   # window within core
    pos_of = np.empty(N, np.int32)    # position within window (0..126)

    for c in range(NCORES):
        lo, hi = NS * c, min(NS * (c + 1), N)
        nodes = np.arange(lo, hi)
        order = nodes[np.argsort(-deg[lo:hi], kind="stable")]
        heap = [(0, 0, w) for w in range(NW)]
        heapq.heapify(heap)
        for n in order:
            load, cnt, w = heapq.heappop(heap)
            slot_of[n] = w
            pos_of[n] = cnt
            cnt += 1
            load += deg[n]
            if cnt < P - 1:  # positions 0..126; 127 reserved for pads
                heapq.heappush(heap, (load, cnt, w))

    # per (core, window, half) edge lists
    core_of = dst // NS
    ed_lists = [[[None, None] for _ in range(NW)] for _ in range(NCORES)]
    for c in range(NCORES):
        m = core_of == c
        s_c, d_c = src[m], dst[m]
        w_c = slot_of[d_c]
        half = (s_c >= SPLIT).astype(np.int64)
        key = w_c * 2 + half
        order = np.argsort(key, kind="stable")
        s_c, d_c, key = s_c[order], d_c[order], key[order]
        bounds = np.searchsorted(key, np.arange(2 * NW + 1))
        for w in range(NW):
            for h in (0, 1):
                a, b = bounds[2 * w + h], bounds[2 * w + h + 1]
                ed_lists[c][w][h] = (s_c[a:b], d_c[a:b])

    # chunk counts per (window, half), equalized across cores
    n_chunks = np.zeros((NW, 2), np.int64)
    for c in range(NCORES):
        for w in range(NW):
            for h in (0, 1):
                n = len(ed_lists[c][w][h][0])
                n_chunks[w, h] = max(n_chunks[w, h], -(-n // P))

    # group metadata (identical across cores)
    groups = []
    lo8_off = hi8_off = ed8_off = col_off = 0
    for g in range(NG):
        wins = list(range(g * G, (g + 1) * G))
        K_lo = int(sum(n_chunks[w, 0] for w in wins))
        K_hi = int(sum(n_chunks[w, 1] for w in wins))
        K = K_lo + K_hi
        win_meta = []
        lo_rel = hi_rel = 0
        for w in wins:
            nl, nh = int(n_chunks[w, 0]), int(n_chunks[w, 1])
            win_meta.append(dict(w=w, lo_rel=lo_rel, n_lo=nl,
                                 hi_rel=hi_rel, n_hi=nh))
            lo_rel += nl
            hi_rel += nh
        groups.append(dict(K_lo=K_lo, K_hi=K_hi, K=K,
                           lo8_off=lo8_off, hi8_off=hi8_off,
                           ed8_off=ed8_off, col_off=col_off,
                           wins=win_meta))
        lo8_off += K_lo * 8
        hi8_off += K_hi * 8
        ed8_off += K * 8
        col_off += K
    LO8, HI8, ED8, KT = lo8_off, hi8_off, ed8_off, col_off

    # per-core packed arrays
    idx_lo = np.zeros((NCORES, 128, LO8), np.int16)
    idx_hi = np.zeros((NCORES, 128, HI8), np.int16)
    idx_ed = np.zeros((NCORES, 128, ED8), np.int16)
    dstl = np.full((NCORES, P, KT), 127.0, np.float32)

    for c in range(NCORES):
        ilo = np.zeros(LO8 * 16, np.int64)   # flat idx pos -> table row
        ihi = np.zeros(HI8 * 16, np.int64)
        ied = np.zeros(ED8 * 16, np.int64)
        for gm in groups:
            for h, (ixarr, base8) in enumerate(
                    ((ilo, gm["lo8_off"]), (ihi, gm["hi8_off"]))):
                sec_rel = 0
                for wm in gm["wins"]:
                    s_e, d_e = ed_lists[c][wm["w"]][h]
                    n = len(s_e)
                    i = np.arange(n)
                    pos = base8 * 16 + (sec_rel + i // P) * P + i % P
                    ixarr[pos] = s_e - (SPLIT if h else 0)
                    # ed idx + dstl live in the unified col space
                    colb = gm["col_off"] + (gm["K_lo"] if h else 0)
                    col = colb + (wm["lo_rel"] if h == 0 else wm["hi_rel"]) \
                        + i // P
                    epos = gm["ed8_off"] * 16 \
                        + ((gm["K_lo"] if h else 0)
                           + (wm["lo_rel"] if h == 0 else wm["hi_rel"])
                           + i // P) * P + i % P
                    ied[epos] = d_e - NS * c
                    dstl[c][i % P, col] = pos_of[d_e]
                    sec_rel += (wm["n_lo"] if h == 0 else wm["n_hi"])
        for flat, arr16 in ((ilo, idx_lo), (ihi, idx_hi), (ied, idx_ed)):
            if flat.size == 0:
                continue
            a = np.zeros((16, flat.size // 16), np.int16)
            a[np.arange(flat.size) % 16, np.arange(flat.size) // 16] = flat
            arr16[c] = np.tile(a, (8, 1))

    return dict(
        groups=groups, LO8=LO8, HI8=HI8, ED8=ED8, KT=KT,
        idx_lo=idx_lo, idx_hi=idx_hi, idx_ed=idx_ed, dstl=dstl,
        slot_of=slot_of, pos_of=pos_of,
    )


def _host_weights(cfg, W, Wb, a, ab):
    """Extended projection weights / bias (o-major, h-inner layout)."""
    H, F, O = W.shape
    a_src, a_dst = a[:, :O], a[:, O:]
    Wmat = W.transpose(1, 2, 0).reshape(F, O * H)          # [F, (o,h)]
    A_s = np.einsum("hfo,ho->fh", W, a_src)
    A_d = np.einsum("hfo,ho->fh", W, a_dst)
    Wext = np.concatenate([Wmat, A_s, A_d], axis=1)        # [F, OH+2H]
    c_s = (Wb * a_src).sum(1)
    c_d = (Wb * a_dst).sum(1)
    bext = np.concatenate([Wb.T.reshape(-1), np.zeros(H, np.float32),
                           c_s + c_d + ab]).astype(np.float32)
    return Wext.astype(np.float32), bext


# --------------------------------------------------------------------------
# Device program
# --------------------------------------------------------------------------

def build_gat_bass(cfg, plan_meta, repeat=1):
    N, F, H, O, NCORES = cfg["N"], cfg["F"], cfg["H"], cfg["O"], cfg["NCORES"]
    NS, NW, G = cfg["NS"], cfg["NW"], cfg["G"]
    HO = H * O
    TDC = HO + H         # useful table row: Wh + e_s = 264
    ROWB = 384           # padded table row elems (768 B)
    EDW = 64             # padded e_d row elems f32 (256 B)
    AD = HO + 2 * H      # phase-A psum width = 272
    NT = NS // P
    KT_F = F // P
    NSG = NS * NCORES

    groups = plan_meta["groups"]
    LO8, HI8, ED8, KT = (plan_meta[k] for k in ("LO8", "HI8", "ED8", "KT"))

    bf = mybir.dt.bfloat16
    f32 = mybir.dt.float32
    i16 = mybir.dt.int16

    nc = bacc.Bacc("TRN2", target_bir_lowering=False, debug=False,
                   num_devices=NCORES, num_swdge_queues=4)

    hT = nc.dram_tensor("hT", [F, NS], bf, kind="ExternalInput")
    wext = nc.dram_tensor("wext", [F, AD], bf, kind="ExternalInput")
    bext = nc.dram_tensor("bext", [1, AD], bf, kind="ExternalInput")
    ones1 = nc.dram_tensor("ones1", [1, P], bf, kind="ExternalInput")
    iota = nc.dram_tensor("iota", [P, P], bf, kind="ExternalInput")
    idx_lo = nc.dram_tensor("idx_lo", [128, LO8], i16, kind="ExternalInput")
    idx_hi = nc.dram_tensor("idx_hi", [128, HI8], i16, kind="ExternalInput")
    idx_ed = nc.dram_tensor("idx_ed", [128, ED8], i16, kind="ExternalInput")
    dstl = nc.dram_tensor("dstl", [P, KT], bf, kind="ExternalInput")

    out_local = nc.dram_tensor("out_local", [NW * P, HO], f32,
                               kind="ExternalOutput")

    with tile.TileContext(nc) as tc:
      for _rep in range(repeat):
        with (
            tc.tile_pool(name="dram", bufs=1, space="DRAM") as dram,
            tc.tile_pool(name="const", bufs=1) as cpool,
        ):
            tbl_local = dram.tile([NS, ROWB], bf)
            tbl_global = dram.tile(
                [NSG, ROWB], bf,
                addr_space="Shared" if NCORES > 4 else "Local")
            ed_local = dram.tile([NS, EDW], f32)

            iota_t = cpool.tile([P, P], bf)
            nc.sync.dma_start(out=iota_t[:], in_=iota[:, :])
            ones_t = cpool.tile([1, P], bf)
            nc.sync.dma_start(out=ones_t[:], in_=ones1[:, :])
            bext_t = cpool.tile([1, AD], bf)
            nc.sync.dma_start(out=bext_t[:], in_=bext[:, :])
            ilo_t = cpool.tile([128, LO8], i16)
            nc.sync.dma_start(out=ilo_t[:], in_=idx_lo[:, :])
            ihi_t = cpool.tile([128, HI8], i16)
            nc.sync.dma_start(out=ihi_t[:], in_=idx_hi[:, :])
            ied_t = cpool.tile([128, ED8], i16)
            nc.sync.dma_start(out=ied_t[:], in_=idx_ed[:, :])
            dstl_t = cpool.tile([P, KT], bf)
            nc.sync.dma_start(out=dstl_t[:], in_=dstl[:, :])

            # ---------------- Phase A: projection ----------------
            with (
                tc.tile_pool(name="pa_sb", bufs=1) as pa,
                tc.tile_pool(name="pa_ps", bufs=2, space="PSUM") as pa_ps,
            ):
                hT_t = pa.tile([P, KT_F * NS], bf, tag="hT")
                for kk in range(KT_F):
                    nc.sync.dma_start(out=hT_t[:, kk * NS:(kk + 1) * NS],
                                      in_=hT[kk * P:(kk + 1) * P, :])
                wext_t = pa.tile([P, KT_F * AD], bf, tag="wext")
                for kk in range(KT_F):
                    nc.sync.dma_start(out=wext_t[:, kk * AD:(kk + 1) * AD],
                                      in_=wext[kk * P:(kk + 1) * P, :])

                stage = pa.tile([P, NT * ROWB], bf, tag="stage")
                nc.vector.memset(stage[:], 0.0)
                ed_stage = pa.tile([P, NT * EDW], f32, tag="ed_stage")
                nc.vector.memset(ed_stage[:], 0.0)

                for t in range(NT):
                    psA = pa_ps.tile([P, AD], f32, tag="psA")
                    for kk in range(KT_F):
                        nc.tensor.matmul(
                            out=psA[:],
                            lhsT=hT_t[:, kk * NS + t * P: kk * NS + (t + 1) * P],
                            rhs=wext_t[:, kk * AD:(kk + 1) * AD],
                            start=(kk == 0), stop=False)
                    nc.tensor.matmul(out=psA[:], lhsT=ones_t[:],
                                     rhs=bext_t[:], start=False, stop=True)
                    nc.vector.tensor_copy(
                        out=stage[:, t * ROWB:t * ROWB + TDC],
                        in_=psA[:, 0:TDC])
                    nc.vector.tensor_copy(
                        out=ed_stage[:, t * EDW:t * EDW + H],
                        in_=psA[:, TDC:TDC + H])

                nc.sync.dma_start(
                    out=tbl_local[:].rearrange("(t p) d -> p t d", p=P),
                    in_=stage[:].rearrange("p (t d) -> p t d", t=NT))
                nc.sync.dma_start(
                    out=ed_local[:].rearrange("(t p) d -> p t d", p=P),
                    in_=ed_stage[:].rearrange("p (t d) -> p t d", t=NT))

            if cfg.get("skip_collective"):
                nc.sync.dma_start(out=tbl_global[0:NS, :], in_=tbl_local[:])
            else:
                nc.gpsimd.collective_compute(
                    "AllGather",
                    mybir.AluOpType.bypass,
                    replica_groups=[list(range(NCORES))],
                    ins=[tbl_local.opt()],
                    outs=[tbl_global.opt()],
                )

            # ---------------- Phase B: edges ----------------
            with (
                tc.tile_pool(name="g_sb", bufs=3) as gp,
                tc.tile_pool(name="ed_sb", bufs=3) as edp,
                tc.tile_pool(name="w_sb", bufs=3) as wpool,
                tc.tile_pool(name="a_sb", bufs=2) as apool,
                tc.tile_pool(name="o_sb", bufs=2) as opool,
                tc.tile_pool(name="agg_ps", bufs=4, space="PSUM") as aggp,
            ):
                # HW limit: a dma_gather's descriptors must fit the SWDGE
                # ring -> cap num_idxs per call (empirically 512 ok, 1920
                # wedges the device). Rotate the 4 SWDGE queues so the next
                # call's descriptor generation overlaps in-flight transfers.
                CAP = int(os.environ.get("GAT_CAP", "7"))
                _q = [0]

                def capped_gather(dst_tile, col0, ncols, in_ap, idx_t,
                                  idx8_off, elem):
                    c, off, left = col0, idx8_off, ncols
                    while left > 0:
                        n = min(left, CAP)
                        nc.gpsimd.dma_gather(
                            dst_tile[:, c * elem:(c + n) * elem].rearrange(
                                "p (k d) -> p k d", k=n),
                            in_ap, idx_t[:, off:off + n * 8],
                            n * P, n * P, elem,
                            queue_num=_q[0])
                        _q[0] = (_q[0] + 1) % 4
                        c += n
                        off += n * 8
                        left -= n

                for gm in groups:
                    K, KL, KH = gm["K"], gm["K_lo"], gm["K_hi"]
                    g_t = gp.tile([P, K * ROWB], bf, tag="g")
                    if KL:
                        capped_gather(g_t, 0, KL, tbl_global[0:SPLIT, :],
                                      ilo_t, gm["lo8_off"], ROWB)
                    if KH:
                        capped_gather(g_t, KL, KH, tbl_global[SPLIT:NSG, :],
                                      ihi_t, gm["hi8_off"], ROWB)
                    ed_t = edp.tile([P, K * EDW], f32, tag="ed")
                    capped_gather(ed_t, 0, K, ed_local[:, :],
                                  ied_t, gm["ed8_off"], EDW)

                    g4 = g_t[:].rearrange("p (k d) -> p k d", k=K)
                    ed4 = ed_t[:].rearrange("p (k d) -> p k d", k=K)

                    # s = e_s + e_d
                    s_t = wpool.tile([P, K * H], f32, tag="s")
                    nc.vector.tensor_tensor(
                        out=s_t[:].rearrange("p (k x) -> p k x", k=K),
                        in0=g4[:, :, HO:TDC],
                        in1=ed4[:, :, 0:H],
                        op=mybir.AluOpType.add)

                    # w = max(exp(s), exp(0.2 s)) -> g cols HO:TDC (bf16)
                    w1 = wpool.tile([P, K * H], f32, tag="w1")
                    nc.scalar.activation(out=w1[:], in_=s_t[:],
                                         func=mybir.ActivationFunctionType.Exp)
                    w2 = wpool.tile([P, K * H], f32, tag="w2")
                    nc.scalar.activation(out=w2[:], in_=s_t[:],
                                         func=mybir.ActivationFunctionType.Exp,
                                         scale=float(cfg["ALPHA"]))
                    nc.vector.tensor_tensor(
                        out=g4[:, :, HO:TDC],
                        in0=w1[:].rearrange("p (k x) -> p k x", k=K),
                        in1=w2[:].rearrange("p (k x) -> p k x", k=K),
                        op=mybir.AluOpType.max)

                    # Wh *= w (broadcast over O), in place
                    nc.vector.tensor_tensor(
                        out=g4[:, :, 0:HO].rearrange(
                            "p k (o x) -> p k o x", o=O),
                        in0=g4[:, :, 0:HO].rearrange(
                            "p k (o x) -> p k o x", o=O),
                        in1=g4[:, :, HO:TDC][:, :, None, :].to_broadcast(
                            [P, K, O, H]),
                        op=mybir.AluOpType.mult)

                    # batched one-hot: a[p, k, v] = (iota[v] == dstl[p, k])
                    a_t = apool.tile([P, K * P], bf, tag="a")
                    nc.vector.tensor_tensor(
                        out=a_t[:].rearrange("p (k v) -> p k v", k=K),
                        in0=iota_t[:][:, None, :].to_broadcast([P, K, P]),
                        in1=dstl_t[:, gm["col_off"]:gm["col_off"] + K][
                            :, :, None].to_broadcast([P, K, P]),
                        op=mybir.AluOpType.is_equal)

                    for wm in gm["wins"]:
                        cols = (list(range(wm["lo_rel"],
                                           wm["lo_rel"] + wm["n_lo"]))
                                + list(range(KL + wm["hi_rel"],
                                             KL + wm["hi_rel"] + wm["n_hi"])))
                        o_t = opool.tile([P, HO], f32, tag="o")
                        if not cols:
                            nc.vector.memset(o_t[:], 0.0)
                        else:
                            agg = aggp.tile([P, TDC], f32, tag="agg")
                            for j, col in enumerate(cols):
                                nc.tensor.matmul(
                                    out=agg[:],
                                    lhsT=a_t[:, col * P:(col + 1) * P],
                                    rhs=g_t[:, col * ROWB:col * ROWB + TDC],
                                    start=(j == 0), stop=(j == len(cols) - 1))
                            den = opool.tile([P, H], f32, tag="den")
                            nc.vector.tensor_scalar(
                                out=den[:], in0=agg[:, HO:TDC],
                                scalar1=1e-30, scalar2=None,
                                op0=mybir.AluOpType.max)
                            rec = opool.tile([P, H], f32, tag="rec")
                            nc.vector.reciprocal(out=rec[:], in_=den[:])
                            nc.vector.tensor_tensor(
                                out=o_t[:].rearrange("p (o x) -> p o x", o=O),
                                in0=agg[:, 0:HO].rearrange(
                                    "p (o x) -> p o x", o=O),
                                in1=rec[:][:, None, :].to_broadcast([P, O, H]),
                                op=mybir.AluOpType.mult)
                        nc.sync.dma_start(
                            out=out_local[wm["w"] * P:(wm["w"] + 1) * P, :],
                            in_=o_t[:])

    return nc


# --------------------------------------------------------------------------
# Execution (PJRT via axon; chained-execution slope timing)
# --------------------------------------------------------------------------

def _run_pjrt_timed(nc, in_maps, n_cores, n_reps=1, chain=None):
    import time

    import jax
    from jax.sharding import Mesh, PartitionSpec
    from jax.experimental.shard_map import shard_map

    from concourse import bass2jax
    from concourse import mybir as mb

    bass2jax.install_neuronx_cc_hook()

    partition_name = (nc.partition_id_tensor.name
                      if nc.partition_id_tensor else None)

    in_names, out_names, out_avals, zero_outs = [], [], [], []
    for alloc in nc.m.functions[0].allocations:
        if not isinstance(alloc, mb.MemoryLocationSet):
            continue
        name = alloc.memorylocations[0].name
        if alloc.kind == "ExternalInput":
            if name != partition_name:
                in_names.append(name)
        elif alloc.kind == "ExternalOutput":
            shape = tuple(alloc.tensor_shape)
            dtype = mb.dt.np(alloc.dtype)
            out_names.append(name)
            out_avals.append(jax.core.ShapedArray(shape, dtype))
            zero_outs.append(np.zeros(shape, dtype))
    n_params = len(in_names)
    n_outs = len(out_avals)
    all_in_names = list(in_names) + out_names
    if partition_name is not None:
        all_in_names.append(partition_name)
    donate = tuple(range(n_params, n_params + n_outs))

    def _body(*args):
        operands = list(args)
        if partition_name is not None:
            operands.append(bass2jax.partition_id_tensor())
        outs = bass2jax._bass_exec_p.bind(
            *operands,
            out_avals=tuple(out_avals),
            in_names=tuple(all_in_names),
            out_names=tuple(out_names),
            lowering_input_output_aliases=(),
            sim_require_finite=True,
            sim_require_nnan=True,
            nc=nc,
        )
        return tuple(outs)

    devices = jax.devices()[:n_cores]
    mesh = Mesh(np.asarray(devices), ("core",))
    in_specs = (PartitionSpec("core"),) * (n_params + n_outs)
    out_specs = (PartitionSpec("core"),) * len(out_names)
    sharded = jax.jit(
        shard_map(_body, mesh=mesh, in_specs=in_specs, out_specs=out_specs,
                  check_rep=False),
        donate_argnums=donate, keep_unused=True)

    sharding = jax.sharding.NamedSharding(mesh, PartitionSpec("core"))
    concat_in = [
        jax.device_put(
            np.concatenate([np.asarray(in_maps[c][name])
                            for c in range(n_cores)], axis=0), sharding)
        for name in in_names
    ]

    def fresh_zeros():
        return [
            jax.device_put(
                np.zeros((n_cores * z.shape[0], *z.shape[1:]), z.dtype),
                sharding)
            for z in zero_outs
        ]

    def run_chain(k):
        outs = fresh_zeros()
        for o in outs:
            o.block_until_ready()
        t0 = time.perf_counter()
        for _ in range(k):
            outs = sharded(*concat_in, *outs)
        for o in outs:
            o.block_until_ready()
        return time.perf_counter() - t0, outs

    # warmup / compile
    _, out_arrs = run_chain(1)

    if chain:
        # Per-execution device time via chained-run slope. Chained calls
        # amortize the fixed dispatch latency; using the min totals at two
        # chain lengths rejects scheduler noise (fixed latency cancels).
        k1, k2 = chain
        t1s, t2s = [], []
        for _ in range(max(1, n_reps)):
            t1, _ = run_chain(k1)
            t2, out_arrs = run_chain(k2)
            t1s.append(t1)
            t2s.append(t2)
        slope = (min(t2s) - min(t1s)) / (k2 - k1)
        _LAST_RESULTS["wall_times_s"] = t1s + t2s
        _LAST_RESULTS["slopes_s"] = [
            (b - a) / (k2 - k1) for a, b in zip(t1s, t2s)]
        _LAST_RESULTS["exec_time_ns"] = int(slope * 1e9)
    else:
        times = []
        for _ in range(max(1, n_reps)):
            t, out_arrs = run_chain(1)
            times.append(t)
        _LAST_RESULTS["wall_times_s"] = times
        _LAST_RESULTS["exec_time_ns"] = int(min(times) * 1e9)

    return [
        {name: np.asarray(out_arrs[i]).reshape(n_cores, *out_avals[i].shape)[c]
         for i, name in enumerate(out_names)}
        for c in range(n_cores)
    ]


def _chain_totals(nc, in_maps, n_cores, k, n_reps):
    """Wall totals of `n_reps` chains of `k` dispatches of nc's program."""
    import time

    import jax
    from jax.sharding import Mesh, PartitionSpec
    from jax.experimental.shard_map import shard_map

    from concourse import bass2jax
    from concourse import mybir as mb

    bass2jax.install_neuronx_cc_hook()
    partition_name = (nc.partition_id_tensor.name
                      if nc.partition_id_tensor else None)
    in_names, out_names, out_avals, zero_outs = [], [], [], []
    for alloc in nc.m.functions[0].allocations:
        if not isinstance(alloc, mb.MemoryLocationSet):
            continue
        name = alloc.memorylocations[0].name
        if alloc.kind == "ExternalInput":
            if name != partition_name:
                in_names.append(name)
        elif alloc.kind == "ExternalOutput":
            shape = tuple(alloc.tensor_shape)
            dtype = mb.dt.np(alloc.dtype)
            out_names.append(name)
            out_avals.append(jax.core.ShapedArray(shape, dtype))
            zero_outs.append(np.zeros(shape, dtype))
    n_params = len(in_names)
    all_in_names = list(in_names) + out_names
    if partition_name is not None:
        all_in_names.append(partition_name)
    donate = tuple(range(n_params, n_params + len(out_avals)))

    def _body(*args):
        operands = list(args)
        if partition_name is not None:
            operands.append(bass2jax.partition_id_tensor())
        return tuple(bass2jax._bass_exec_p.bind(
            *operands, out_avals=tuple(out_avals),
            in_names=tuple(all_in_names), out_names=tuple(out_names),
            lowering_input_output_aliases=(),
            sim_require_finite=True, sim_require_nnan=True, nc=nc))

    devices = jax.devices()[:n_cores]
    mesh = Mesh(np.asarray(devices), ("core",))
    nio = n_params + len(out_avals)
    sharded = jax.jit(
        shard_map(_body, mesh=mesh, in_specs=(PartitionSpec("core"),) * nio,
                  out_specs=(PartitionSpec("core"),) * len(out_names),
                  check_rep=False),
        donate_argnums=donate, keep_unused=True)
    sharding = jax.sharding.NamedSharding(mesh, PartitionSpec("core"))
    concat_in = [
        jax.device_put(
            np.concatenate([np.asarray(in_maps[c][name])
                            for c in range(n_cores)], axis=0), sharding)
        for name in in_names
    ]

    def run_chain(kk):
        outs = [jax.device_put(
            np.zeros((n_cores * z.shape[0], *z.shape[1:]), z.dtype), sharding)
            for z in zero_outs]
        for o in outs:
            o.block_until_ready()
        t0 = time.perf_counter()
        for _ in range(kk):
            outs = sharded(*concat_in, *outs)
        for o in outs:
            o.block_until_ready()
        return time.perf_counter() - t0

    run_chain(1)  # warmup/compile
    return [run_chain(k) for _ in range(n_reps)]


# --------------------------------------------------------------------------
# Host entry point
# --------------------------------------------------------------------------

def _run(cfg, h, src, dst, W, Wb, a, ab, use_sim=False, timing=False):
    N, F, H, O, NCORES = cfg["N"], cfg["F"], cfg["H"], cfg["O"], cfg["NCORES"]
    NS, NW = cfg["NS"], cfg["NW"]
    HO = H * O

    h = np.asarray(h, np.float32)
    src = np.asarray(src).astype(np.int64)
    dst = np.asarray(dst).astype(np.int64)
    W = np.asarray(W, np.float32)
    Wb = np.asarray(Wb, np.float32)
    a = np.asarray(a, np.float32)
    ab = np.asarray(ab, np.float32)

    plan = _plan(cfg, src, dst)
    Wext, bext = _host_weights(cfg, W, Wb, a, ab)

    NSG = NS * NCORES
    h_pad = np.zeros((NSG, F), np.float32)
    h_pad[:N] = h
    iota_np = np.broadcast_to(np.arange(P, dtype=np.float32),
                              (P, P)).astype(BF16)
    in_maps = []
    for c in range(NCORES):
        in_maps.append({
            "hT": np.ascontiguousarray(
                h_pad[NS * c:NS * (c + 1)].T).astype(BF16),
            "wext": Wext.astype(BF16),
            "bext": bext.reshape(1, -1).astype(BF16),
            "ones1": np.ones((1, P), BF16),
            "iota": iota_np,
            "idx_lo": plan["idx_lo"][c],
            "idx_hi": plan["idx_hi"][c],
            "idx_ed": plan["idx_ed"][c],
            "dstl": plan["dstl"][c].astype(BF16),
        })

    nc = build_gat_bass(cfg, plan)
    nc.compile()

    if use_sim:
        from concourse import bass_interp
        sim = bass_interp.MultiCoreSim(nc, NCORES)
        for c in range(NCORES):
            for k, v in in_maps[c].items():
                sim.cores[c].tensor(k)[:] = v
        sim.simulate()
        outs = [np.array(sim.cores[c].mem_tensor("out_local"))
                for c in range(NCORES)]
    else:
        results = _run_pjrt_timed(nc, in_maps, NCORES, n_reps=1)
        outs = [results[c]["out_local"] for c in range(NCORES)]
        if timing:
            # Device-time measurement: the per-dispatch overhead of this
            # PJRT/axon path (~1.5 ms, measured with a trivial kernel) dwarfs
            # the kernel, so time a variant program whose body runs REP times
            # per dispatch and take the slope over REP via chained runs at
            # two chain lengths (min over reps cancels dispatch noise).
            REP, K, NR = cfg.get("REP", 5), 24, 3
            ncR = build_gat_bass(cfg, plan, repeat=REP)
            ncR.compile()
            t1s = _chain_totals(nc, in_maps, NCORES, K, NR)
            t2s = _chain_totals(ncR, in_maps, NCORES, K, NR)
            d = (min(t2s) - min(t1s)) / (K * (REP - 1))
            _LAST_RESULTS["wall_times_s"] = t1s + t2s
            _LAST_RESULTS["exec_time_ns"] = int(d * 1e9)

    # unscramble rows + columns
    slot_of, pos_of = plan["slot_of"], plan["pos_of"]
    nodes = np.arange(N)
    rows = slot_of[nodes] * P + pos_of[nodes]
    out = np.empty((N, HO), np.float32)
    for c in range(NCORES):
        lo, hi = NS * c, min(NS * (c + 1), N)
        out[lo:hi] = outs[c][rows[lo:hi]]
    # column map: ref col h*O+o <- ours o*H+h
    hh, oo = np.meshgrid(np.arange(H), np.arange(O), indexing="ij")
    colmap = (oo * H + hh).reshape(-1)
    return out[:, colmap]


def kernel(h, src, dst, W, Wb, a, ab):
    cfg = dict(FULL_CFG)
    timing = os.environ.get("GAT_TRACE", "0") == "1"
    use_sim = os.environ.get("GAT_SIM", "0") == "1"
    return _run(cfg, h, src, dst, W, Wb, a, ab, use_sim=use_sim,
                timing=timing)


# revision 4
# speedup vs baseline: 1.1194x; 1.0338x over previous
"""GAT (graph attention) layer on 8 Trainium2 NeuronCores — v2.

Strategy (dst-partitioned edge parallelism, bulk SWDGE gathers):
  - Nodes split into 8 contiguous row-ranges (one per core). Per core,
    dst nodes are packed into NW windows of <=127 nodes.
  - Phase A (per core): project the core's node shard with TensorE:
        [Wh | e_s | e_d] = h_shard @ [Wmat | A_s | A_d] + bias
    Rows [Wh | e_s] (264 bf16, padded to 384 = 768B) go to a DRAM table
    that is AllGathered so every core holds all 50176 rows; e_d rows
    (8 f32, padded to 64 = 256B) stay in a core-local DRAM mini-table.
  - Phase B (per core): edges grouped by destination window; G windows
    form a "group". Per group, three bulk dma_gather calls fetch all
    per-edge data (dma_gather's int16 indices only address 32768 rows,
    so the global table is read through two base offsets — "lo" rows
    [0, 32768) and "hi" rows [32768, NSG) — and every 128-edge chunk is
    packed all-lo or all-hi on the host):
      * table rows by src (one call per lo/hi section)
      * e_d rows by dst from the local mini-table (single call; dst is
        always in the local shard)
    Then per group: w = max(exp(e_s+e_d), exp(0.2(e_s+e_d))) (ACT+DVE),
    one batched DVE op builds all 128-edge one-hot matrices, and per
    window C matmuls accumulate [sum w*Wh | sum w] in PSUM, normalized
    once per window (equivalent to the reference softmax; scores are
    bounded so the max-subtraction is unnecessary).
  - Host does only index/layout work: edge bucketing, packing,
    transposes, and final row/column unscrambles.
"""
import os
import sys

sys.path.insert(0, "/opt/trn_rl_repo")

import numpy as np
import ml_dtypes

import concourse.bass as bass
import concourse.bacc as bacc
import concourse.tile as tile
import concourse.mybir as mybir

BF16 = np.dtype(ml_dtypes.bfloat16)
P = 128
SPLIT = 32768          # dma_gather int16 index limit

FULL_CFG = dict(
    N=50000, F=512, H=8, O=32, ALPHA=0.2, NCORES=8,
    NS=6272, NW=50, G=2,
)

_LAST_RESULTS = {}  # exposed for test.py (exec time etc.)


# --------------------------------------------------------------------------
# Host-side planning
# --------------------------------------------------------------------------

def _plan(cfg, src, dst):
    """Window assignment + lo/hi chunk packing, equalized across cores.

    Returns per-core index arrays (shared shapes) + group metadata
    (identical across cores, baked into the SPMD program).
    """
    import heapq

    N, NCORES = cfg["N"], cfg["NCORES"]
    NS, NW, G = cfg["NS"], cfg["NW"], cfg["G"]
    NG = NW // G
    assert NW % G == 0

    deg = np.bincount(dst, minlength=N).astype(np.int64)

    slot_of = np.empty(N, np.int32)   # window within core
    pos_of = np.empty(N, np.int32)    # position within window (0..126)

    for c in range(NCORES):
        lo, hi = NS * c, min(NS * (c + 1), N)
        nodes = np.arange(lo, hi)
        order = nodes[np.argsort(-deg[lo:hi], kind="stable")]
        heap = [(0, 0, w) for w in range(NW)]
        heapq.heapify(heap)
        for n in order:
            load, cnt, w = heapq.heappop(heap)
            slot_of[n] = w
            pos_of[n] = cnt
            cnt += 1
            load += deg[n]
            if cnt < P - 1:  # positions 0..126; 127 reserved for pads
                heapq.heappush(heap, (load, cnt, w))

    # per (core, window, half) edge lists
    core_of = dst // NS
    ed_lists = [[[None, None] for _ in range(NW)] for _ in range(NCORES)]
    for c in range(NCORES):
        m = core_of == c
        s_c, d_c = src[m], dst[m]
        w_c = slot_of[d_c]
        half = (s_c >= SPLIT).astype(np.int64)
        key = w_c * 2 + half
        order = np.argsort(key, kind="stable")
        s_c, d_c, key = s_c[order], d_c[order], key[order]
        bounds = np.searchsorted(key, np.arange(2 * NW + 1))
        for w in range(NW):
            for h in (0, 1):
                a, b = bounds[2 * w + h], bounds[2 * w + h + 1]
                ed_lists[c][w][h] = (s_c[a:b], d_c[a:b])

    # chunk counts per (window, half), equalized across cores
    n_chunks = np.zeros((NW, 2), np.int64)
    for c in range(NCORES):
        for w in range(NW):
            for h in (0, 1):
                n = len(ed_lists[c][w][h][0])
                n_chunks[w, h] = max(n_chunks[w, h], -(-n // P))

    # group metadata (identical across cores)
    groups = []
    lo8_off = hi8_off = ed8_off = col_off = 0
    for g in range(NG):
        wins = list(range(g * G, (g + 1) * G))
        K_lo = int(sum(n_chunks[w, 0] for w in wins))
        K_hi = int(sum(n_chunks[w, 1] for w in wins))
        K = K_lo + K_hi
        win_meta = []
        lo_rel = hi_rel = 0
        for w in wins:
            nl, nh = int(n_chunks[w, 0]), int(n_chunks[w, 1])
            win_meta.append(dict(w=w, lo_rel=lo_rel, n_lo=nl,
                                 hi_rel=hi_rel, n_hi=nh))
            lo_rel += nl
            hi_rel += nh
        groups.append(dict(K_lo=K_lo, K_hi=K_hi, K=K,
                           lo8_off=lo8_off, hi8_off=hi8_off,
                           ed8_off=ed8_off, col_off=col_off,
                           wins=win_meta))
        lo8_off += K_lo * 8
        hi8_off += K_hi * 8
        ed8_off += K * 8
        col_off += K
    LO8, HI8, ED8, KT = lo8_off, hi8_off, ed8_off, col_off

    # per-core packed arrays
    idx_lo = np.zeros((NCORES, 128, LO8), np.int16)
    idx_hi = np.zeros((NCORES, 128, HI8), np.int16)
    idx_ed = np.zeros((NCORES, 128, ED8), np.int16)
    dstl = np.full((NCORES, P, KT), 127.0, np.float32)

    for c in range(NCORES):
        ilo = np.zeros(LO8 * 16, np.int64)   # flat idx pos -> table row
        ihi = np.zeros(HI8 * 16, np.int64)
        ied = np.zeros(ED8 * 16, np.int64)
        for gm in groups:
            for h, (ixarr, base8) in enumerate(
                    ((ilo, gm["lo8_off"]), (ihi, gm["hi8_off"]))):
                sec_rel = 0
                for wm in gm["wins"]:
                    s_e, d_e = ed_lists[c][wm["w"]][h]
                    n = len(s_e)
                    i = np.arange(n)
                    pos = base8 * 16 + (sec_rel + i // P) * P + i % P
                    ixarr[pos] = s_e - (SPLIT if h else 0)
                    # ed idx + dstl live in the unified col space
                    colb = gm["col_off"] + (gm["K_lo"] if h else 0)
                    col = colb + (wm["lo_rel"] if h == 0 else wm["hi_rel"]) \
                        + i // P
                    epos = gm["ed8_off"] * 16 \
                        + ((gm["K_lo"] if h else 0)
                           + (wm["lo_rel"] if h == 0 else wm["hi_rel"])
                           + i // P) * P + i % P
                    ied[epos] = d_e - NS * c
                    dstl[c][i % P, col] = pos_of[d_e]
                    sec_rel += (wm["n_lo"] if h == 0 else wm["n_hi"])
        for flat, arr16 in ((ilo, idx_lo), (ihi, idx_hi), (ied, idx_ed)):
            if flat.size == 0:
                continue
            a = np.zeros((16, flat.size // 16), np.int16)
            a[np.arange(flat.size) % 16, np.arange(flat.size) // 16] = flat
            arr16[c] = np.tile(a, (8, 1))

    return dict(
        groups=groups, LO8=LO8, HI8=HI8, ED8=ED8, KT=KT,
        idx_lo=idx_lo, idx_hi=idx_hi, idx_ed=idx_ed, dstl=dstl,
        slot_of=slot_of, pos_of=pos_of,
    )


def _host_weights(cfg, W, Wb, a, ab):
    """Extended projection weights / bias (o-major, h-inner layout)."""
    H, F, O = W.shape
    a_src, a_dst = a[:, :O], a[:, O:]
    Wmat = W.transpose(1, 2, 0).reshape(F, O * H)          # [F, (o,h)]
    A_s = np.einsum("hfo,ho->fh", W, a_src)
    A_d = np.einsum("hfo,ho->fh", W, a_dst)
    Wext = np.concatenate([Wmat, A_s, A_d], axis=1)        # [F, OH+2H]
    c_s = (Wb * a_src).sum(1)
    c_d = (Wb * a_dst).sum(1)
    bext = np.concatenate([Wb.T.reshape(-1), np.zeros(H, np.float32),
                           c_s + c_d + ab]).astype(np.float32)
    return Wext.astype(np.float32), bext


# --------------------------------------------------------------------------
# Device program
# --------------------------------------------------------------------------

def build_gat_bass(cfg, plan_meta, repeat=1):
    N, F, H, O, NCORES = cfg["N"], cfg["F"], cfg["H"], cfg["O"], cfg["NCORES"]
    NS, NW, G = cfg["NS"], cfg["NW"], cfg["G"]
    HO = H * O
    TDC = HO + H         # useful table row: Wh + e_s = 264
    ROWB = 384           # padded table row elems (768 B)
    EDW = 64             # padded e_d row elems f32 (256 B)
    AD = HO + 2 * H      # phase-A psum width = 272
    NT = NS // P
    KT_F = F // P
    NSG = NS * NCORES

    groups = plan_meta["groups"]
    LO8, HI8, ED8, KT = (plan_meta[k] for k in ("LO8", "HI8", "ED8", "KT"))

    bf = mybir.dt.bfloat16
    f32 = mybir.dt.float32
    i16 = mybir.dt.int16

    nc = bacc.Bacc("TRN2", target_bir_lowering=False, debug=False,
                   num_devices=NCORES, num_swdge_queues=4)

    hT = nc.dram_tensor("hT", [F, NS], bf, kind="ExternalInput")
    wext = nc.dram_tensor("wext", [F, AD], bf, kind="ExternalInput")
    bext = nc.dram_tensor("bext", [1, AD], bf, kind="ExternalInput")
    ones1 = nc.dram_tensor("ones1", [1, P], bf, kind="ExternalInput")
    iota = nc.dram_tensor("iota", [P, P], bf, kind="ExternalInput")
    idx_lo = nc.dram_tensor("idx_lo", [128, LO8], i16, kind="ExternalInput")
    idx_hi = nc.dram_tensor("idx_hi", [128, HI8], i16, kind="ExternalInput")
    idx_ed = nc.dram_tensor("idx_ed", [128, ED8], i16, kind="ExternalInput")
    dstl = nc.dram_tensor("dstl", [P, KT], bf, kind="ExternalInput")

    out_local = nc.dram_tensor("out_local", [NW * P, HO], f32,
                               kind="ExternalOutput")

    with tile.TileContext(nc) as tc:
      for _rep in range(repeat):
        with (
            tc.tile_pool(name="dram", bufs=1, space="DRAM") as dram,
            tc.tile_pool(name="const", bufs=1) as cpool,
        ):
            tbl_local = dram.tile([NS, ROWB], bf)
            tbl_global = dram.tile(
                [NSG, ROWB], bf,
                addr_space="Shared" if NCORES > 4 else "Local")
            ed_local = dram.tile([NS, EDW], f32)

            iota_t = cpool.tile([P, P], bf)
            nc.sync.dma_start(out=iota_t[:], in_=iota[:, :])
            ones_t = cpool.tile([1, P], bf)
            nc.sync.dma_start(out=ones_t[:], in_=ones1[:, :])
            bext_t = cpool.tile([1, AD], bf)
            nc.sync.dma_start(out=bext_t[:], in_=bext[:, :])
            ilo_t = cpool.tile([128, LO8], i16)
            nc.sync.dma_start(out=ilo_t[:], in_=idx_lo[:, :])
            ihi_t = cpool.tile([128, HI8], i16)
            nc.sync.dma_start(out=ihi_t[:], in_=idx_hi[:, :])
            ied_t = cpool.tile([128, ED8], i16)
            nc.sync.dma_start(out=ied_t[:], in_=idx_ed[:, :])
            dstl_t = cpool.tile([P, KT], bf)
            nc.sync.dma_start(out=dstl_t[:], in_=dstl[:, :])

            # ---------------- Phase A: projection ----------------
            with (
                tc.tile_pool(name="pa_sb", bufs=1) as pa,
                tc.tile_pool(name="pa_ps", bufs=2, space="PSUM") as pa_ps,
            ):
                hT_t = pa.tile([P, KT_F * NS], bf, tag="hT")
                for kk in range(KT_F):
                    nc.sync.dma_start(out=hT_t[:, kk * NS:(kk + 1) * NS],
                                      in_=hT[kk * P:(kk + 1) * P, :])
                wext_t = pa.tile([P, KT_F * AD], bf, tag="wext")
                for kk in range(KT_F):
                    nc.sync.dma_start(out=wext_t[:, kk * AD:(kk + 1) * AD],
                                      in_=wext[kk * P:(kk + 1) * P, :])

                stage = pa.tile([P, NT * ROWB], bf, tag="stage")
                nc.vector.memset(stage[:], 0.0)
                ed_stage = pa.tile([P, NT * EDW], f32, tag="ed_stage")
                nc.vector.memset(ed_stage[:], 0.0)

                for t in range(NT):
                    psA = pa_ps.tile([P, AD], f32, tag="psA")
                    for kk in range(KT_F):
                        nc.tensor.matmul(
                            out=psA[:],
                            lhsT=hT_t[:, kk * NS + t * P: kk * NS + (t + 1) * P],
                            rhs=wext_t[:, kk * AD:(kk + 1) * AD],
                            start=(kk == 0), stop=False)
                    nc.tensor.matmul(out=psA[:], lhsT=ones_t[:],
                                     rhs=bext_t[:], start=False, stop=True)
                    nc.vector.tensor_copy(
                        out=stage[:, t * ROWB:t * ROWB + TDC],
                        in_=psA[:, 0:TDC])
                    nc.vector.tensor_copy(
                        out=ed_stage[:, t * EDW:t * EDW + H],
                        in_=psA[:, TDC:TDC + H])

                nc.sync.dma_start(
                    out=tbl_local[:].rearrange("(t p) d -> p t d", p=P),
                    in_=stage[:].rearrange("p (t d) -> p t d", t=NT))
                nc.sync.dma_start(
                    out=ed_local[:].rearrange("(t p) d -> p t d", p=P),
                    in_=ed_stage[:].rearrange("p (t d) -> p t d", t=NT))

            if cfg.get("skip_collective"):
                nc.sync.dma_start(out=tbl_global[0:NS, :], in_=tbl_local[:])
            else:
                nc.gpsimd.collective_compute(
                    "AllGather",
                    mybir.AluOpType.bypass,
                    replica_groups=[list(range(NCORES))],
                    ins=[tbl_local.opt()],
                    outs=[tbl_global.opt()],
                )

            # ---------------- Phase B: edges ----------------
            with (
                tc.tile_pool(name="g_sb", bufs=3) as gp,
                tc.tile_pool(name="ed_sb", bufs=3) as edp,
                tc.tile_pool(name="w_sb", bufs=3) as wpool,
                tc.tile_pool(name="a_sb", bufs=2) as apool,
                tc.tile_pool(name="o_sb", bufs=2) as opool,
                tc.tile_pool(name="agg_ps", bufs=4, space="PSUM") as aggp,
            ):
                # HW limit: a dma_gather's descriptors must fit the SWDGE
                # ring -> cap num_idxs per call (empirically 512 ok, 1920
                # wedges the device). Rotate the 4 SWDGE queues so the next
                # call's descriptor generation overlaps in-flight transfers.
                CAP = int(os.environ.get("GAT_CAP", "7"))
                _q = [0]

                def capped_gather(dst_tile, col0, ncols, in_ap, idx_t,
                                  idx8_off, elem):
                    c, off, left = col0, idx8_off, ncols
                    while left > 0:
                        n = min(left, CAP)
                        nc.gpsimd.dma_gather(
                            dst_tile[:, c * elem:(c + n) * elem].rearrange(
                                "p (k d) -> p k d", k=n),
                            in_ap, idx_t[:, off:off + n * 8],
                            n * P, n * P, elem,
                            queue_num=_q[0])
                        _q[0] = (_q[0] + 1) % 4
                        c += n
                        off += n * 8
                        left -= n

                for gm in groups:
                    K, KL, KH = gm["K"], gm["K_lo"], gm["K_hi"]
                    # ed gather first: it only depends on phase A's local
                    # write, so it can proceed while the AllGather (which
                    # gates the table gathers) is still in flight.
                    ed_t = edp.tile([P, K * EDW], f32, tag="ed")
                    capped_gather(ed_t, 0, K, ed_local[:, :],
                                  ied_t, gm["ed8_off"], EDW)
                    g_t = gp.tile([P, K * ROWB], bf, tag="g")
                    if KL:
                        capped_gather(g_t, 0, KL, tbl_global[0:SPLIT, :],
                                      ilo_t, gm["lo8_off"], ROWB)
                    if KH:
                        capped_gather(g_t, KL, KH, tbl_global[SPLIT:NSG, :],
                                      ihi_t, gm["hi8_off"], ROWB)

                    g4 = g_t[:].rearrange("p (k d) -> p k d", k=K)
                    ed4 = ed_t[:].rearrange("p (k d) -> p k d", k=K)

                    # s = e_s + e_d
                    s_t = wpool.tile([P, K * H], f32, tag="s")
                    nc.vector.tensor_tensor(
                        out=s_t[:].rearrange("p (k x) -> p k x", k=K),
                        in0=g4[:, :, HO:TDC],
                        in1=ed4[:, :, 0:H],
                        op=mybir.AluOpType.add)

                    # w = max(exp(s), exp(0.2 s)) -> g cols HO:TDC (bf16)
                    w1 = wpool.tile([P, K * H], f32, tag="w1")
                    nc.scalar.activation(out=w1[:], in_=s_t[:],
                                         func=mybir.ActivationFunctionType.Exp)
                    w2 = wpool.tile([P, K * H], f32, tag="w2")
                    nc.scalar.activation(out=w2[:], in_=s_t[:],
                                         func=mybir.ActivationFunctionType.Exp,
                                         scale=float(cfg["ALPHA"]))
                    nc.vector.tensor_tensor(
                        out=g4[:, :, HO:TDC],
                        in0=w1[:].rearrange("p (k x) -> p k x", k=K),
                        in1=w2[:].rearrange("p (k x) -> p k x", k=K),
                        op=mybir.AluOpType.max)

                    # Wh *= w (broadcast over O), in place
                    nc.vector.tensor_tensor(
                        out=g4[:, :, 0:HO].rearrange(
                            "p k (o x) -> p k o x", o=O),
                        in0=g4[:, :, 0:HO].rearrange(
                            "p k (o x) -> p k o x", o=O),
                        in1=g4[:, :, HO:TDC][:, :, None, :].to_broadcast(
                            [P, K, O, H]),
                        op=mybir.AluOpType.mult)

                    # batched one-hot: a[p, k, v] = (iota[v] == dstl[p, k])
                    a_t = apool.tile([P, K * P], bf, tag="a")
                    nc.vector.tensor_tensor(
                        out=a_t[:].rearrange("p (k v) -> p k v", k=K),
                        in0=iota_t[:][:, None, :].to_broadcast([P, K, P]),
                        in1=dstl_t[:, gm["col_off"]:gm["col_off"] + K][
                            :, :, None].to_broadcast([P, K, P]),
                        op=mybir.AluOpType.is_equal)

                    for wm in gm["wins"]:
                        cols = (list(range(wm["lo_rel"],
                                           wm["lo_rel"] + wm["n_lo"]))
                                + list(range(KL + wm["hi_rel"],
                                             KL + wm["hi_rel"] + wm["n_hi"])))
                        o_t = opool.tile([P, HO], f32, tag="o")
                        if not cols:
                            nc.vector.memset(o_t[:], 0.0)
                        else:
                            agg = aggp.tile([P, TDC], f32, tag="agg")
                            for j, col in enumerate(cols):
                                nc.tensor.matmul(
                                    out=agg[:],
                                    lhsT=a_t[:, col * P:(col + 1) * P],
                                    rhs=g_t[:, col * ROWB:col * ROWB + TDC],
                                    start=(j == 0), stop=(j == len(cols) - 1))
                            den = opool.tile([P, H], f32, tag="den")
                            nc.vector.tensor_scalar(
                                out=den[:], in0=agg[:, HO:TDC],
                                scalar1=1e-30, scalar2=None,
                                op0=mybir.AluOpType.max)
                            rec = opool.tile([P, H], f32, tag="rec")
                            nc.vector.reciprocal(out=rec[:], in_=den[:])
                            nc.vector.tensor_tensor(
                                out=o_t[:].rearrange("p (o x) -> p o x", o=O),
                                in0=agg[:, 0:HO].rearrange(
                                    "p (o x) -> p o x", o=O),
                                in1=rec[:][:, None, :].to_broadcast([P, O, H]),
                                op=mybir.AluOpType.mult)
                        nc.sync.dma_start(
                            out=out_local[wm["w"] * P:(wm["w"] + 1) * P, :],
                            in_=o_t[:])

    return nc


# --------------------------------------------------------------------------
# Execution (PJRT via axon; chained-execution slope timing)
# --------------------------------------------------------------------------

def _run_pjrt_timed(nc, in_maps, n_cores, n_reps=1, chain=None):
    import time

    import jax
    from jax.sharding import Mesh, PartitionSpec
    from jax.experimental.shard_map import shard_map

    from concourse import bass2jax
    from concourse import mybir as mb

    bass2jax.install_neuronx_cc_hook()

    partition_name = (nc.partition_id_tensor.name
                      if nc.partition_id_tensor else None)

    in_names, out_names, out_avals, zero_outs = [], [], [], []
    for alloc in nc.m.functions[0].allocations:
        if not isinstance(alloc, mb.MemoryLocationSet):
            continue
        name = alloc.memorylocations[0].name
        if alloc.kind == "ExternalInput":
            if name != partition_name:
                in_names.append(name)
        elif alloc.kind == "ExternalOutput":
            shape = tuple(alloc.tensor_shape)
            dtype = mb.dt.np(alloc.dtype)
            out_names.append(name)
            out_avals.append(jax.core.ShapedArray(shape, dtype))
            zero_outs.append(np.zeros(shape, dtype))
    n_params = len(in_names)
    n_outs = len(out_avals)
    all_in_names = list(in_names) + out_names
    if partition_name is not None:
        all_in_names.append(partition_name)
    donate = tuple(range(n_params, n_params + n_outs))

    def _body(*args):
        operands = list(args)
        if partition_name is not None:
            operands.append(bass2jax.partition_id_tensor())
        outs = bass2jax._bass_exec_p.bind(
            *operands,
            out_avals=tuple(out_avals),
            in_names=tuple(all_in_names),
            out_names=tuple(out_names),
            lowering_input_output_aliases=(),
            sim_require_finite=True,
            sim_require_nnan=True,
            nc=nc,
        )
        return tuple(outs)

    devices = jax.devices()[:n_cores]
    mesh = Mesh(np.asarray(devices), ("core",))
    in_specs = (PartitionSpec("core"),) * (n_params + n_outs)
    out_specs = (PartitionSpec("core"),) * len(out_names)
    sharded = jax.jit(
        shard_map(_body, mesh=mesh, in_specs=in_specs, out_specs=out_specs,
                  check_rep=False),
        donate_argnums=donate, keep_unused=True)

    sharding = jax.sharding.NamedSharding(mesh, PartitionSpec("core"))
    concat_in = [
        jax.device_put(
            np.concatenate([np.asarray(in_maps[c][name])
                            for c in range(n_cores)], axis=0), sharding)
        for name in in_names
    ]

    def fresh_zeros():
        return [
            jax.device_put(
                np.zeros((n_cores * z.shape[0], *z.shape[1:]), z.dtype),
                sharding)
            for z in zero_outs
        ]

    def run_chain(k):
        outs = fresh_zeros()
        for o in outs:
            o.block_until_ready()
        t0 = time.perf_counter()
        for _ in range(k):
            outs = sharded(*concat_in, *outs)
        for o in outs:
            o.block_until_ready()
        return time.perf_counter() - t0, outs

    # warmup / compile
    _, out_arrs = run_chain(1)

    if chain:
        # Per-execution device time via chained-run slope. Chained calls
        # amortize the fixed dispatch latency; using the min totals at two
        # chain lengths rejects scheduler noise (fixed latency cancels).
        k1, k2 = chain
        t1s, t2s = [], []
        for _ in range(max(1, n_reps)):
            t1, _ = run_chain(k1)
            t2, out_arrs = run_chain(k2)
            t1s.append(t1)
            t2s.append(t2)
        slope = (min(t2s) - min(t1s)) / (k2 - k1)
        _LAST_RESULTS["wall_times_s"] = t1s + t2s
        _LAST_RESULTS["slopes_s"] = [
            (b - a) / (k2 - k1) for a, b in zip(t1s, t2s)]
        _LAST_RESULTS["exec_time_ns"] = int(slope * 1e9)
    else:
        times = []
        for _ in range(max(1, n_reps)):
            t, out_arrs = run_chain(1)
            times.append(t)
        _LAST_RESULTS["wall_times_s"] = times
        _LAST_RESULTS["exec_time_ns"] = int(min(times) * 1e9)

    return [
        {name: np.asarray(out_arrs[i]).reshape(n_cores, *out_avals[i].shape)[c]
         for i, name in enumerate(out_names)}
        for c in range(n_cores)
    ]


def _chain_totals(nc, in_maps, n_cores, k, n_reps):
    """Wall totals of `n_reps` chains of `k` dispatches of nc's program."""
    import time

    import jax
    from jax.sharding import Mesh, PartitionSpec
    from jax.experimental.shard_map import shard_map

    from concourse import bass2jax
    from concourse import mybir as mb

    bass2jax.install_neuronx_cc_hook()
    partition_name = (nc.partition_id_tensor.name
                      if nc.partition_id_tensor else None)
    in_names, out_names, out_avals, zero_outs = [], [], [], []
    for alloc in nc.m.functions[0].allocations:
        if not isinstance(alloc, mb.MemoryLocationSet):
            continue
        name = alloc.memorylocations[0].name
        if alloc.kind == "ExternalInput":
            if name != partition_name:
                in_names.append(name)
        elif alloc.kind == "ExternalOutput":
            shape = tuple(alloc.tensor_shape)
            dtype = mb.dt.np(alloc.dtype)
            out_names.append(name)
            out_avals.append(jax.core.ShapedArray(shape, dtype))
            zero_outs.append(np.zeros(shape, dtype))
    n_params = len(in_names)
    all_in_names = list(in_names) + out_names
    if partition_name is not None:
        all_in_names.append(partition_name)
    donate = tuple(range(n_params, n_params + len(out_avals)))

    def _body(*args):
        operands = list(args)
        if partition_name is not None:
            operands.append(bass2jax.partition_id_tensor())
        return tuple(bass2jax._bass_exec_p.bind(
            *operands, out_avals=tuple(out_avals),
            in_names=tuple(all_in_names), out_names=tuple(out_names),
            lowering_input_output_aliases=(),
            sim_require_finite=True, sim_require_nnan=True, nc=nc))

    devices = jax.devices()[:n_cores]
    mesh = Mesh(np.asarray(devices), ("core",))
    nio = n_params + len(out_avals)
    sharded = jax.jit(
        shard_map(_body, mesh=mesh, in_specs=(PartitionSpec("core"),) * nio,
                  out_specs=(PartitionSpec("core"),) * len(out_names),
                  check_rep=False),
        donate_argnums=donate, keep_unused=True)
    sharding = jax.sharding.NamedSharding(mesh, PartitionSpec("core"))
    concat_in = [
        jax.device_put(
            np.concatenate([np.asarray(in_maps[c][name])
                            for c in range(n_cores)], axis=0), sharding)
        for name in in_names
    ]

    def run_chain(kk):
        outs = [jax.device_put(
            np.zeros((n_cores * z.shape[0], *z.shape[1:]), z.dtype), sharding)
            for z in zero_outs]
        for o in outs:
            o.block_until_ready()
        t0 = time.perf_counter()
        for _ in range(kk):
            outs = sharded(*concat_in, *outs)
        for o in outs:
            o.block_until_ready()
        return time.perf_counter() - t0

    run_chain(1)  # warmup/compile
    return [run_chain(k) for _ in range(n_reps)]


# --------------------------------------------------------------------------
# Host entry point
# --------------------------------------------------------------------------

def _run(cfg, h, src, dst, W, Wb, a, ab, use_sim=False, timing=False):
    N, F, H, O, NCORES = cfg["N"], cfg["F"], cfg["H"], cfg["O"], cfg["NCORES"]
    NS, NW = cfg["NS"], cfg["NW"]
    HO = H * O

    h = np.asarray(h, np.float32)
    src = np.asarray(src).astype(np.int64)
    dst = np.asarray(dst).astype(np.int64)
    W = np.asarray(W, np.float32)
    Wb = np.asarray(Wb, np.float32)
    a = np.asarray(a, np.float32)
    ab = np.asarray(ab, np.float32)

    plan = _plan(cfg, src, dst)
    Wext, bext = _host_weights(cfg, W, Wb, a, ab)

    NSG = NS * NCORES
    h_pad = np.zeros((NSG, F), np.float32)
    h_pad[:N] = h
    iota_np = np.broadcast_to(np.arange(P, dtype=np.float32),
                              (P, P)).astype(BF16)
    in_maps = []
    for c in range(NCORES):
        in_maps.append({
            "hT": np.ascontiguousarray(
                h_pad[NS * c:NS * (c + 1)].T).astype(BF16),
            "wext": Wext.astype(BF16),
            "bext": bext.reshape(1, -1).astype(BF16),
            "ones1": np.ones((1, P), BF16),
            "iota": iota_np,
            "idx_lo": plan["idx_lo"][c],
            "idx_hi": plan["idx_hi"][c],
            "idx_ed": plan["idx_ed"][c],
            "dstl": plan["dstl"][c].astype(BF16),
        })

    nc = build_gat_bass(cfg, plan)
    nc.compile()

    if use_sim:
        from concourse import bass_interp
        sim = bass_interp.MultiCoreSim(nc, NCORES)
        for c in range(NCORES):
            for k, v in in_maps[c].items():
                sim.cores[c].tensor(k)[:] = v
        sim.simulate()
        outs = [np.array(sim.cores[c].mem_tensor("out_local"))
                for c in range(NCORES)]
    else:
        results = _run_pjrt_timed(nc, in_maps, NCORES, n_reps=1)
        outs = [results[c]["out_local"] for c in range(NCORES)]
        if timing:
            # Device-time measurement: the per-dispatch overhead of this
            # PJRT/axon path (~1.5 ms, measured with a trivial kernel) dwarfs
            # the kernel, so time a variant program whose body runs REP times
            # per dispatch and take the slope over REP via chained runs at
            # two chain lengths (min over reps cancels dispatch noise).
            REP, K, NR = cfg.get("REP", 5), 24, 3
            ncR = build_gat_bass(cfg, plan, repeat=REP)
            ncR.compile()
            t1s = _chain_totals(nc, in_maps, NCORES, K, NR)
            t2s = _chain_totals(ncR, in_maps, NCORES, K, NR)
            d = (min(t2s) - min(t1s)) / (K * (REP - 1))
            _LAST_RESULTS["wall_times_s"] = t1s + t2s
            _LAST_RESULTS["exec_time_ns"] = int(d * 1e9)

    # unscramble rows + columns
    slot_of, pos_of = plan["slot_of"], plan["pos_of"]
    nodes = np.arange(N)
    rows = slot_of[nodes] * P + pos_of[nodes]
    out = np.empty((N, HO), np.float32)
    for c in range(NCORES):
        lo, hi = NS * c, min(NS * (c + 1), N)
        out[lo:hi] = outs[c][rows[lo:hi]]
    # column map: ref col h*O+o <- ours o*H+h
    hh, oo = np.meshgrid(np.arange(H), np.arange(O), indexing="ij")
    colmap = (oo * H + hh).reshape(-1)
    return out[:, colmap]


def kernel(h, src, dst, W, Wb, a, ab):
    cfg = dict(FULL_CFG)
    timing = os.environ.get("GAT_TRACE", "0") == "1"
    use_sim = os.environ.get("GAT_SIM", "0") == "1"
    return _run(cfg, h, src, dst, W, Wb, a, ab, use_sim=use_sim,
                timing=timing)


# revision 5
# speedup vs baseline: 1.1611x; 1.0373x over previous
"""GAT (graph attention) layer on 8 Trainium2 NeuronCores — v2.

Strategy (dst-partitioned edge parallelism, bulk SWDGE gathers):
  - Nodes split into 8 contiguous row-ranges (one per core). Per core,
    dst nodes are packed into NW windows of <=127 nodes.
  - Phase A (per core): project the core's node shard with TensorE:
        [Wh | e_s | e_d] = h_shard @ [Wmat | A_s | A_d] + bias
    Rows [Wh | e_s] (264 bf16, padded to 384 = 768B) go to a DRAM table
    that is AllGathered so every core holds all 50176 rows; e_d rows
    (8 f32, padded to 64 = 256B) stay in a core-local DRAM mini-table.
  - Phase B (per core): edges grouped by destination window; G windows
    form a "group". Per group, three bulk dma_gather calls fetch all
    per-edge data (dma_gather's int16 indices only address 32768 rows,
    so the global table is read through two base offsets — "lo" rows
    [0, 32768) and "hi" rows [32768, NSG) — and every 128-edge chunk is
    packed all-lo or all-hi on the host):
      * table rows by src (one call per lo/hi section)
      * e_d rows by dst from the local mini-table (single call; dst is
        always in the local shard)
    Then per group: w = max(exp(e_s+e_d), exp(0.2(e_s+e_d))) (ACT+DVE),
    one batched DVE op builds all 128-edge one-hot matrices, and per
    window C matmuls accumulate [sum w*Wh | sum w] in PSUM, normalized
    once per window (equivalent to the reference softmax; scores are
    bounded so the max-subtraction is unnecessary).
  - Host does only index/layout work: edge bucketing, packing,
    transposes, and final row/column unscrambles.
"""
import os
import sys

sys.path.insert(0, "/opt/trn_rl_repo")

import numpy as np
import ml_dtypes

import concourse.bass as bass
import concourse.bacc as bacc
import concourse.tile as tile
import concourse.mybir as mybir

BF16 = np.dtype(ml_dtypes.bfloat16)
P = 128
SPLIT = 32768          # dma_gather int16 index limit

FULL_CFG = dict(
    N=50000, F=512, H=8, O=32, ALPHA=0.2, NCORES=8,
    NS=6272, NW=50, G=2,
)

_LAST_RESULTS = {}  # exposed for test.py (exec time etc.)


# --------------------------------------------------------------------------
# Host-side planning
# --------------------------------------------------------------------------

def _plan(cfg, src, dst):
    """Window assignment + lo/hi chunk packing, equalized across cores.

    Returns per-core index arrays (shared shapes) + group metadata
    (identical across cores, baked into the SPMD program).
    """
    import heapq

    N, NCORES = cfg["N"], cfg["NCORES"]
    NS, NW, G = cfg["NS"], cfg["NW"], cfg["G"]
    NG = NW // G
    assert NW % G == 0

    deg = np.bincount(dst, minlength=N).astype(np.int64)

    slot_of = np.empty(N, np.int32)   # window within core
    pos_of = np.empty(N, np.int32)    # position within window (0..126)

    for c in range(NCORES):
        lo, hi = NS * c, min(NS * (c + 1), N)
        nodes = np.arange(lo, hi)
        order = nodes[np.argsort(-deg[lo:hi], kind="stable")]
        heap = [(0, 0, w) for w in range(NW)]
        heapq.heapify(heap)
        for n in order:
            load, cnt, w = heapq.heappop(heap)
            slot_of[n] = w
            pos_of[n] = cnt
            cnt += 1
            load += deg[n]
            if cnt < P - 1:  # positions 0..126; 127 reserved for pads
                heapq.heappush(heap, (load, cnt, w))

    # per (core, window, half) edge lists
    core_of = dst // NS
    ed_lists = [[[None, None] for _ in range(NW)] for _ in range(NCORES)]
    for c in range(NCORES):
        m = core_of == c
        s_c, d_c = src[m], dst[m]
        w_c = slot_of[d_c]
        half = (s_c >= SPLIT).astype(np.int64)
        key = w_c * 2 + half
        order = np.argsort(key, kind="stable")
        s_c, d_c, key = s_c[order], d_c[order], key[order]
        bounds = np.searchsorted(key, np.arange(2 * NW + 1))
        for w in range(NW):
            for h in (0, 1):
                a, b = bounds[2 * w + h], bounds[2 * w + h + 1]
                ed_lists[c][w][h] = (s_c[a:b], d_c[a:b])

    # chunk counts per (window, half), equalized across cores
    n_chunks = np.zeros((NW, 2), np.int64)
    for c in range(NCORES):
        for w in range(NW):
            for h in (0, 1):
                n = len(ed_lists[c][w][h][0])
                n_chunks[w, h] = max(n_chunks[w, h], -(-n // P))

    # group metadata (identical across cores)
    groups = []
    lo8_off = hi8_off = ed8_off = col_off = 0
    for g in range(NG):
        wins = list(range(g * G, (g + 1) * G))
        K_lo = int(sum(n_chunks[w, 0] for w in wins))
        K_hi = int(sum(n_chunks[w, 1] for w in wins))
        K = K_lo + K_hi
        win_meta = []
        lo_rel = hi_rel = 0
        for w in wins:
            nl, nh = int(n_chunks[w, 0]), int(n_chunks[w, 1])
            win_meta.append(dict(w=w, lo_rel=lo_rel, n_lo=nl,
                                 hi_rel=hi_rel, n_hi=nh))
            lo_rel += nl
            hi_rel += nh
        groups.append(dict(K_lo=K_lo, K_hi=K_hi, K=K,
                           lo8_off=lo8_off, hi8_off=hi8_off,
                           ed8_off=ed8_off, col_off=col_off,
                           wins=win_meta))
        lo8_off += K_lo * 8
        hi8_off += K_hi * 8
        ed8_off += K * 8
        col_off += K
    LO8, HI8, ED8, KT = lo8_off, hi8_off, ed8_off, col_off

    # per-core packed arrays
    idx_lo = np.zeros((NCORES, 128, LO8), np.int16)
    idx_hi = np.zeros((NCORES, 128, HI8), np.int16)
    idx_ed = np.zeros((NCORES, 128, ED8), np.int16)
    dstl = np.full((NCORES, P, KT), 127.0, np.float32)

    for c in range(NCORES):
        ilo = np.zeros(LO8 * 16, np.int64)   # flat idx pos -> table row
        ihi = np.zeros(HI8 * 16, np.int64)
        ied = np.zeros(ED8 * 16, np.int64)
        for gm in groups:
            for h, (ixarr, base8) in enumerate(
                    ((ilo, gm["lo8_off"]), (ihi, gm["hi8_off"]))):
                sec_rel = 0
                for wm in gm["wins"]:
                    s_e, d_e = ed_lists[c][wm["w"]][h]
                    n = len(s_e)
                    i = np.arange(n)
                    pos = base8 * 16 + (sec_rel + i // P) * P + i % P
                    ixarr[pos] = s_e - (SPLIT if h else 0)
                    # ed idx + dstl live in the unified col space
                    colb = gm["col_off"] + (gm["K_lo"] if h else 0)
                    col = colb + (wm["lo_rel"] if h == 0 else wm["hi_rel"]) \
                        + i // P
                    epos = gm["ed8_off"] * 16 \
                        + ((gm["K_lo"] if h else 0)
                           + (wm["lo_rel"] if h == 0 else wm["hi_rel"])
                           + i // P) * P + i % P
                    ied[epos] = d_e - NS * c
                    dstl[c][i % P, col] = pos_of[d_e]
                    sec_rel += (wm["n_lo"] if h == 0 else wm["n_hi"])
        for flat, arr16 in ((ilo, idx_lo), (ihi, idx_hi), (ied, idx_ed)):
            if flat.size == 0:
                continue
            a = np.zeros((16, flat.size // 16), np.int16)
            a[np.arange(flat.size) % 16, np.arange(flat.size) // 16] = flat
            arr16[c] = np.tile(a, (8, 1))

    return dict(
        groups=groups, LO8=LO8, HI8=HI8, ED8=ED8, KT=KT,
        idx_lo=idx_lo, idx_hi=idx_hi, idx_ed=idx_ed, dstl=dstl,
        slot_of=slot_of, pos_of=pos_of,
    )


def _host_weights(cfg, W, Wb, a, ab):
    """Extended projection weights / bias (o-major, h-inner layout)."""
    H, F, O = W.shape
    a_src, a_dst = a[:, :O], a[:, O:]
    Wmat = W.transpose(1, 2, 0).reshape(F, O * H)          # [F, (o,h)]
    A_s = np.einsum("hfo,ho->fh", W, a_src)
    A_d = np.einsum("hfo,ho->fh", W, a_dst)
    Wext = np.concatenate([Wmat, A_s, A_d], axis=1)        # [F, OH+2H]
    c_s = (Wb * a_src).sum(1)
    c_d = (Wb * a_dst).sum(1)
    bext = np.concatenate([Wb.T.reshape(-1), np.zeros(H, np.float32),
                           c_s + c_d + ab]).astype(np.float32)
    return Wext.astype(np.float32), bext


# --------------------------------------------------------------------------
# Device program
# --------------------------------------------------------------------------

def build_gat_bass(cfg, plan_meta, repeat=1):
    N, F, H, O, NCORES = cfg["N"], cfg["F"], cfg["H"], cfg["O"], cfg["NCORES"]
    NS, NW, G = cfg["NS"], cfg["NW"], cfg["G"]
    HO = H * O
    TDC = HO + H         # useful table row: Wh + e_s = 264
    ROWB = 384           # padded table row elems (768 B)
    EDW = 64             # padded e_d row elems f32 (256 B)
    AD = HO + 2 * H      # phase-A psum width = 272
    NT = NS // P
    KT_F = F // P
    NSG = NS * NCORES

    groups = plan_meta["groups"]
    LO8, HI8, ED8, KT = (plan_meta[k] for k in ("LO8", "HI8", "ED8", "KT"))

    bf = mybir.dt.bfloat16
    f32 = mybir.dt.float32
    i16 = mybir.dt.int16

    nc = bacc.Bacc("TRN2", target_bir_lowering=False, debug=False,
                   num_devices=NCORES, num_swdge_queues=4)

    hT = nc.dram_tensor("hT", [F, NS], bf, kind="ExternalInput")
    wext = nc.dram_tensor("wext", [F, AD], bf, kind="ExternalInput")
    bext = nc.dram_tensor("bext", [1, AD], bf, kind="ExternalInput")
    ones1 = nc.dram_tensor("ones1", [1, P], bf, kind="ExternalInput")
    iota = nc.dram_tensor("iota", [P, P], bf, kind="ExternalInput")
    idx_lo = nc.dram_tensor("idx_lo", [128, LO8], i16, kind="ExternalInput")
    idx_hi = nc.dram_tensor("idx_hi", [128, HI8], i16, kind="ExternalInput")
    idx_ed = nc.dram_tensor("idx_ed", [128, ED8], i16, kind="ExternalInput")
    dstl = nc.dram_tensor("dstl", [P, KT], bf, kind="ExternalInput")

    out_local = nc.dram_tensor("out_local", [NW * P, HO], f32,
                               kind="ExternalOutput")

    with tile.TileContext(nc) as tc:
      for _rep in range(repeat):
        with (
            tc.tile_pool(name="dram", bufs=1, space="DRAM") as dram,
            tc.tile_pool(name="const", bufs=1) as cpool,
        ):
            tbl_local = dram.tile([NS, ROWB], bf)
            tbl_global = dram.tile(
                [NSG, ROWB], bf,
                addr_space="Shared" if NCORES > 4 else "Local")
            ed_local = dram.tile([NS, EDW], f32)

            iota_t = cpool.tile([P, P], bf)
            nc.sync.dma_start(out=iota_t[:], in_=iota[:, :])
            ones_t = cpool.tile([1, P], bf)
            nc.sync.dma_start(out=ones_t[:], in_=ones1[:, :])
            bext_t = cpool.tile([1, AD], bf)
            nc.sync.dma_start(out=bext_t[:], in_=bext[:, :])
            ilo_t = cpool.tile([128, LO8], i16)
            nc.sync.dma_start(out=ilo_t[:], in_=idx_lo[:, :])
            ihi_t = cpool.tile([128, HI8], i16)
            nc.sync.dma_start(out=ihi_t[:], in_=idx_hi[:, :])
            ied_t = cpool.tile([128, ED8], i16)
            nc.sync.dma_start(out=ied_t[:], in_=idx_ed[:, :])
            dstl_t = cpool.tile([P, KT], bf)
            nc.sync.dma_start(out=dstl_t[:], in_=dstl[:, :])

            # ---------------- Phase A: projection ----------------
            with (
                tc.tile_pool(name="pa_sb", bufs=1) as pa,
                tc.tile_pool(name="pa_ps", bufs=2, space="PSUM") as pa_ps,
            ):
                hT_t = pa.tile([P, KT_F * NS], bf, tag="hT")
                for kk in range(KT_F):
                    nc.sync.dma_start(out=hT_t[:, kk * NS:(kk + 1) * NS],
                                      in_=hT[kk * P:(kk + 1) * P, :])
                wext_t = pa.tile([P, KT_F * AD], bf, tag="wext")
                for kk in range(KT_F):
                    nc.sync.dma_start(out=wext_t[:, kk * AD:(kk + 1) * AD],
                                      in_=wext[kk * P:(kk + 1) * P, :])

                stage = pa.tile([P, NT * TDC], bf, tag="stage")
                ed_stage = pa.tile([P, NT * H], f32, tag="ed_stage")

                for t in range(NT):
                    psA = pa_ps.tile([P, AD], f32, tag="psA")
                    for kk in range(KT_F):
                        nc.tensor.matmul(
                            out=psA[:],
                            lhsT=hT_t[:, kk * NS + t * P: kk * NS + (t + 1) * P],
                            rhs=wext_t[:, kk * AD:(kk + 1) * AD],
                            start=(kk == 0), stop=False)
                    nc.tensor.matmul(out=psA[:], lhsT=ones_t[:],
                                     rhs=bext_t[:], start=False, stop=True)
                    nc.vector.tensor_copy(
                        out=stage[:, t * TDC:(t + 1) * TDC],
                        in_=psA[:, 0:TDC])
                    nc.vector.tensor_copy(
                        out=ed_stage[:, t * H:(t + 1) * H],
                        in_=psA[:, TDC:TDC + H])

                # write only the live 264/8 leading columns of each padded
                # row; DRAM pad columns stay uninitialized (gathered into
                # unused SBUF columns, never read by compute)
                nc.sync.dma_start(
                    out=tbl_local[:].rearrange(
                        "(t p) d -> p t d", p=P)[:, :, 0:TDC],
                    in_=stage[:].rearrange("p (t d) -> p t d", t=NT))
                nc.sync.dma_start(
                    out=ed_local[:].rearrange(
                        "(t p) d -> p t d", p=P)[:, :, 0:H],
                    in_=ed_stage[:].rearrange("p (t d) -> p t d", t=NT))

            if cfg.get("skip_collective"):
                nc.sync.dma_start(out=tbl_global[0:NS, :], in_=tbl_local[:])
            else:
                nc.gpsimd.collective_compute(
                    "AllGather",
                    mybir.AluOpType.bypass,
                    replica_groups=[list(range(NCORES))],
                    ins=[tbl_local.opt()],
                    outs=[tbl_global.opt()],
                )

            # ---------------- Phase B: edges ----------------
            with (
                tc.tile_pool(name="g_sb", bufs=3) as gp,
                tc.tile_pool(name="ed_sb", bufs=3) as edp,
                tc.tile_pool(name="w_sb", bufs=3) as wpool,
                tc.tile_pool(name="a_sb", bufs=2) as apool,
                tc.tile_pool(name="o_sb", bufs=3) as opool,
                tc.tile_pool(name="agg_ps", bufs=6, space="PSUM") as aggp,
            ):
                # HW limit: a dma_gather's descriptors must fit the SWDGE
                # ring -> cap num_idxs per call (empirically 512 ok, 1920
                # wedges the device). Rotate the 4 SWDGE queues so the next
                # call's descriptor generation overlaps in-flight transfers.
                CAP = int(os.environ.get("GAT_CAP", "7"))
                _q = [0]

                def capped_gather(dst_tile, col0, ncols, in_ap, idx_t,
                                  idx8_off, elem):
                    c, off, left = col0, idx8_off, ncols
                    while left > 0:
                        n = min(left, CAP)
                        nc.gpsimd.dma_gather(
                            dst_tile[:, c * elem:(c + n) * elem].rearrange(
                                "p (k d) -> p k d", k=n),
                            in_ap, idx_t[:, off:off + n * 8],
                            n * P, n * P, elem,
                            queue_num=_q[0])
                        _q[0] = (_q[0] + 1) % 4
                        c += n
                        off += n * 8
                        left -= n

                for gm in groups:
                    K, KL, KH = gm["K"], gm["K_lo"], gm["K_hi"]
                    # ed gather first: it only depends on phase A's local
                    # write, so it can proceed while the AllGather (which
                    # gates the table gathers) is still in flight.
                    ed_t = edp.tile([P, K * EDW], f32, tag="ed")
                    capped_gather(ed_t, 0, K, ed_local[:, :],
                                  ied_t, gm["ed8_off"], EDW)
                    g_t = gp.tile([P, K * ROWB], bf, tag="g")
                    if KL:
                        capped_gather(g_t, 0, KL, tbl_global[0:SPLIT, :],
                                      ilo_t, gm["lo8_off"], ROWB)
                    if KH:
                        capped_gather(g_t, KL, KH, tbl_global[SPLIT:NSG, :],
                                      ihi_t, gm["hi8_off"], ROWB)

                    g4 = g_t[:].rearrange("p (k d) -> p k d", k=K)
                    ed4 = ed_t[:].rearrange("p (k d) -> p k d", k=K)

                    # s = e_s + e_d
                    s_t = wpool.tile([P, K * H], f32, tag="s")
                    nc.vector.tensor_tensor(
                        out=s_t[:].rearrange("p (k x) -> p k x", k=K),
                        in0=g4[:, :, HO:TDC],
                        in1=ed4[:, :, 0:H],
                        op=mybir.AluOpType.add)

                    # w = max(exp(s), exp(0.2 s)) -> g cols HO:TDC (bf16)
                    w1 = wpool.tile([P, K * H], f32, tag="w1")
                    nc.scalar.activation(out=w1[:], in_=s_t[:],
                                         func=mybir.ActivationFunctionType.Exp)
                    w2 = wpool.tile([P, K * H], f32, tag="w2")
                    nc.scalar.activation(out=w2[:], in_=s_t[:],
                                         func=mybir.ActivationFunctionType.Exp,
                                         scale=float(cfg["ALPHA"]))
                    nc.vector.tensor_tensor(
                        out=g4[:, :, HO:TDC],
                        in0=w1[:].rearrange("p (k x) -> p k x", k=K),
                        in1=w2[:].rearrange("p (k x) -> p k x", k=K),
                        op=mybir.AluOpType.max)

                    # Wh *= w (broadcast over O), in place
                    nc.vector.tensor_tensor(
                        out=g4[:, :, 0:HO].rearrange(
                            "p k (o x) -> p k o x", o=O),
                        in0=g4[:, :, 0:HO].rearrange(
                            "p k (o x) -> p k o x", o=O),
                        in1=g4[:, :, HO:TDC][:, :, None, :].to_broadcast(
                            [P, K, O, H]),
                        op=mybir.AluOpType.mult)

                    # batched one-hot: a[p, k, v] = (iota[v] == dstl[p, k])
                    a_t = apool.tile([P, K * P], bf, tag="a")
                    nc.vector.tensor_tensor(
                        out=a_t[:].rearrange("p (k v) -> p k v", k=K),
                        in0=iota_t[:][:, None, :].to_broadcast([P, K, P]),
                        in1=dstl_t[:, gm["col_off"]:gm["col_off"] + K][
                            :, :, None].to_broadcast([P, K, P]),
                        op=mybir.AluOpType.is_equal)

                    for wm in gm["wins"]:
                        cols = (list(range(wm["lo_rel"],
                                           wm["lo_rel"] + wm["n_lo"]))
                                + list(range(KL + wm["hi_rel"],
                                             KL + wm["hi_rel"] + wm["n_hi"])))
                        o_t = opool.tile([P, HO], f32, tag="o")
                        if not cols:
                            nc.vector.memset(o_t[:], 0.0)
                        else:
                            agg = aggp.tile([P, TDC], f32, tag="agg")
                            for j, col in enumerate(cols):
                                nc.tensor.matmul(
                                    out=agg[:],
                                    lhsT=a_t[:, col * P:(col + 1) * P],
                                    rhs=g_t[:, col * ROWB:col * ROWB + TDC],
                                    start=(j == 0), stop=(j == len(cols) - 1))
                            den = opool.tile([P, H], f32, tag="den")
                            nc.vector.tensor_scalar(
                                out=den[:], in0=agg[:, HO:TDC],
                                scalar1=1e-30, scalar2=None,
                                op0=mybir.AluOpType.max)
                            rec = opool.tile([P, H], f32, tag="rec")
                            nc.vector.reciprocal(out=rec[:], in_=den[:])
                            nc.vector.tensor_tensor(
                                out=o_t[:].rearrange("p (o x) -> p o x", o=O),
                                in0=agg[:, 0:HO].rearrange(
                                    "p (o x) -> p o x", o=O),
                                in1=rec[:][:, None, :].to_broadcast([P, O, H]),
                                op=mybir.AluOpType.mult)
                        nc.sync.dma_start(
                            out=out_local[wm["w"] * P:(wm["w"] + 1) * P, :],
                            in_=o_t[:])

    return nc


# --------------------------------------------------------------------------
# Execution (PJRT via axon; chained-execution slope timing)
# --------------------------------------------------------------------------

def _run_pjrt_timed(nc, in_maps, n_cores, n_reps=1, chain=None):
    import time

    import jax
    from jax.sharding import Mesh, PartitionSpec
    from jax.experimental.shard_map import shard_map

    from concourse import bass2jax
    from concourse import mybir as mb

    bass2jax.install_neuronx_cc_hook()

    partition_name = (nc.partition_id_tensor.name
                      if nc.partition_id_tensor else None)

    in_names, out_names, out_avals, zero_outs = [], [], [], []
    for alloc in nc.m.functions[0].allocations:
        if not isinstance(alloc, mb.MemoryLocationSet):
            continue
        name = alloc.memorylocations[0].name
        if alloc.kind == "ExternalInput":
            if name != partition_name:
                in_names.append(name)
        elif alloc.kind == "ExternalOutput":
            shape = tuple(alloc.tensor_shape)
            dtype = mb.dt.np(alloc.dtype)
            out_names.append(name)
            out_avals.append(jax.core.ShapedArray(shape, dtype))
            zero_outs.append(np.zeros(shape, dtype))
    n_params = len(in_names)
    n_outs = len(out_avals)
    all_in_names = list(in_names) + out_names
    if partition_name is not None:
        all_in_names.append(partition_name)
    donate = tuple(range(n_params, n_params + n_outs))

    def _body(*args):
        operands = list(args)
        if partition_name is not None:
            operands.append(bass2jax.partition_id_tensor())
        outs = bass2jax._bass_exec_p.bind(
            *operands,
            out_avals=tuple(out_avals),
            in_names=tuple(all_in_names),
            out_names=tuple(out_names),
            lowering_input_output_aliases=(),
            sim_require_finite=True,
            sim_require_nnan=True,
            nc=nc,
        )
        return tuple(outs)

    devices = jax.devices()[:n_cores]
    mesh = Mesh(np.asarray(devices), ("core",))
    in_specs = (PartitionSpec("core"),) * (n_params + n_outs)
    out_specs = (PartitionSpec("core"),) * len(out_names)
    sharded = jax.jit(
        shard_map(_body, mesh=mesh, in_specs=in_specs, out_specs=out_specs,
                  check_rep=False),
        donate_argnums=donate, keep_unused=True)

    sharding = jax.sharding.NamedSharding(mesh, PartitionSpec("core"))
    concat_in = [
        jax.device_put(
            np.concatenate([np.asarray(in_maps[c][name])
                            for c in range(n_cores)], axis=0), sharding)
        for name in in_names
    ]

    def fresh_zeros():
        return [
            jax.device_put(
                np.zeros((n_cores * z.shape[0], *z.shape[1:]), z.dtype),
                sharding)
            for z in zero_outs
        ]

    def run_chain(k):
        outs = fresh_zeros()
        for o in outs:
            o.block_until_ready()
        t0 = time.perf_counter()
        for _ in range(k):
            outs = sharded(*concat_in, *outs)
        for o in outs:
            o.block_until_ready()
        return time.perf_counter() - t0, outs

    # warmup / compile
    _, out_arrs = run_chain(1)

    if chain:
        # Per-execution device time via chained-run slope. Chained calls
        # amortize the fixed dispatch latency; using the min totals at two
        # chain lengths rejects scheduler noise (fixed latency cancels).
        k1, k2 = chain
        t1s, t2s = [], []
        for _ in range(max(1, n_reps)):
            t1, _ = run_chain(k1)
            t2, out_arrs = run_chain(k2)
            t1s.append(t1)
            t2s.append(t2)
        slope = (min(t2s) - min(t1s)) / (k2 - k1)
        _LAST_RESULTS["wall_times_s"] = t1s + t2s
        _LAST_RESULTS["slopes_s"] = [
            (b - a) / (k2 - k1) for a, b in zip(t1s, t2s)]
        _LAST_RESULTS["exec_time_ns"] = int(slope * 1e9)
    else:
        times = []
        for _ in range(max(1, n_reps)):
            t, out_arrs = run_chain(1)
            times.append(t)
        _LAST_RESULTS["wall_times_s"] = times
        _LAST_RESULTS["exec_time_ns"] = int(min(times) * 1e9)

    return [
        {name: np.asarray(out_arrs[i]).reshape(n_cores, *out_avals[i].shape)[c]
         for i, name in enumerate(out_names)}
        for c in range(n_cores)
    ]


def _chain_totals(nc, in_maps, n_cores, k, n_reps):
    """Wall totals of `n_reps` chains of `k` dispatches of nc's program."""
    import time

    import jax
    from jax.sharding import Mesh, PartitionSpec
    from jax.experimental.shard_map import shard_map

    from concourse import bass2jax
    from concourse import mybir as mb

    bass2jax.install_neuronx_cc_hook()
    partition_name = (nc.partition_id_tensor.name
                      if nc.partition_id_tensor else None)
    in_names, out_names, out_avals, zero_outs = [], [], [], []
    for alloc in nc.m.functions[0].allocations:
        if not isinstance(alloc, mb.MemoryLocationSet):
            continue
        name = alloc.memorylocations[0].name
        if alloc.kind == "ExternalInput":
            if name != partition_name:
                in_names.append(name)
        elif alloc.kind == "ExternalOutput":
            shape = tuple(alloc.tensor_shape)
            dtype = mb.dt.np(alloc.dtype)
            out_names.append(name)
            out_avals.append(jax.core.ShapedArray(shape, dtype))
            zero_outs.append(np.zeros(shape, dtype))
    n_params = len(in_names)
    all_in_names = list(in_names) + out_names
    if partition_name is not None:
        all_in_names.append(partition_name)
    donate = tuple(range(n_params, n_params + len(out_avals)))

    def _body(*args):
        operands = list(args)
        if partition_name is not None:
            operands.append(bass2jax.partition_id_tensor())
        return tuple(bass2jax._bass_exec_p.bind(
            *operands, out_avals=tuple(out_avals),
            in_names=tuple(all_in_names), out_names=tuple(out_names),
            lowering_input_output_aliases=(),
            sim_require_finite=True, sim_require_nnan=True, nc=nc))

    devices = jax.devices()[:n_cores]
    mesh = Mesh(np.asarray(devices), ("core",))
    nio = n_params + len(out_avals)
    sharded = jax.jit(
        shard_map(_body, mesh=mesh, in_specs=(PartitionSpec("core"),) * nio,
                  out_specs=(PartitionSpec("core"),) * len(out_names),
                  check_rep=False),
        donate_argnums=donate, keep_unused=True)
    sharding = jax.sharding.NamedSharding(mesh, PartitionSpec("core"))
    concat_in = [
        jax.device_put(
            np.concatenate([np.asarray(in_maps[c][name])
                            for c in range(n_cores)], axis=0), sharding)
        for name in in_names
    ]

    def run_chain(kk):
        outs = [jax.device_put(
            np.zeros((n_cores * z.shape[0], *z.shape[1:]), z.dtype), sharding)
            for z in zero_outs]
        for o in outs:
            o.block_until_ready()
        t0 = time.perf_counter()
        for _ in range(kk):
            outs = sharded(*concat_in, *outs)
        for o in outs:
            o.block_until_ready()
        return time.perf_counter() - t0

    run_chain(1)  # warmup/compile
    return [run_chain(k) for _ in range(n_reps)]


# --------------------------------------------------------------------------
# Host entry point
# --------------------------------------------------------------------------

def _run(cfg, h, src, dst, W, Wb, a, ab, use_sim=False, timing=False):
    N, F, H, O, NCORES = cfg["N"], cfg["F"], cfg["H"], cfg["O"], cfg["NCORES"]
    NS, NW = cfg["NS"], cfg["NW"]
    HO = H * O

    h = np.asarray(h, np.float32)
    src = np.asarray(src).astype(np.int64)
    dst = np.asarray(dst).astype(np.int64)
    W = np.asarray(W, np.float32)
    Wb = np.asarray(Wb, np.float32)
    a = np.asarray(a, np.float32)
    ab = np.asarray(ab, np.float32)

    plan = _plan(cfg, src, dst)
    Wext, bext = _host_weights(cfg, W, Wb, a, ab)

    NSG = NS * NCORES
    h_pad = np.zeros((NSG, F), np.float32)
    h_pad[:N] = h
    iota_np = np.broadcast_to(np.arange(P, dtype=np.float32),
                              (P, P)).astype(BF16)
    in_maps = []
    for c in range(NCORES):
        in_maps.append({
            "hT": np.ascontiguousarray(
                h_pad[NS * c:NS * (c + 1)].T).astype(BF16),
            "wext": Wext.astype(BF16),
            "bext": bext.reshape(1, -1).astype(BF16),
            "ones1": np.ones((1, P), BF16),
            "iota": iota_np,
            "idx_lo": plan["idx_lo"][c],
            "idx_hi": plan["idx_hi"][c],
            "idx_ed": plan["idx_ed"][c],
            "dstl": plan["dstl"][c].astype(BF16),
        })

    nc = build_gat_bass(cfg, plan)
    nc.compile()

    if use_sim:
        from concourse import bass_interp
        sim = bass_interp.MultiCoreSim(nc, NCORES)
        for c in range(NCORES):
            for k, v in in_maps[c].items():
                sim.cores[c].tensor(k)[:] = v
        sim.simulate()
        outs = [np.array(sim.cores[c].mem_tensor("out_local"))
                for c in range(NCORES)]
    else:
        results = _run_pjrt_timed(nc, in_maps, NCORES, n_reps=1)
        outs = [results[c]["out_local"] for c in range(NCORES)]
        if timing:
            # Device-time measurement: the per-dispatch overhead of this
            # PJRT/axon path (~1.5 ms, measured with a trivial kernel) dwarfs
            # the kernel, so time a variant program whose body runs REP times
            # per dispatch and take the slope over REP via chained runs at
            # two chain lengths (min over reps cancels dispatch noise).
            REP, K, NR = cfg.get("REP", 5), 24, 3
            ncR = build_gat_bass(cfg, plan, repeat=REP)
            ncR.compile()
            t1s = _chain_totals(nc, in_maps, NCORES, K, NR)
            t2s = _chain_totals(ncR, in_maps, NCORES, K, NR)
            d = (min(t2s) - min(t1s)) / (K * (REP - 1))
            _LAST_RESULTS["wall_times_s"] = t1s + t2s
            _LAST_RESULTS["exec_time_ns"] = int(d * 1e9)

    # unscramble rows + columns
    slot_of, pos_of = plan["slot_of"], plan["pos_of"]
    nodes = np.arange(N)
    rows = slot_of[nodes] * P + pos_of[nodes]
    out = np.empty((N, HO), np.float32)
    for c in range(NCORES):
        lo, hi = NS * c, min(NS * (c + 1), N)
        out[lo:hi] = outs[c][rows[lo:hi]]
    # column map: ref col h*O+o <- ours o*H+h
    hh, oo = np.meshgrid(np.arange(H), np.arange(O), indexing="ij")
    colmap = (oo * H + hh).reshape(-1)
    return out[:, colmap]


def kernel(h, src, dst, W, Wb, a, ab):
    cfg = dict(FULL_CFG)
    timing = os.environ.get("GAT_TRACE", "0") == "1"
    use_sim = os.environ.get("GAT_SIM", "0") == "1"
    return _run(cfg, h, src, dst, W, Wb, a, ab, use_sim=use_sim,
                timing=timing)


# revision 7
# speedup vs baseline: 1.2067x; 1.0393x over previous
"""GAT (graph attention) layer on 8 Trainium2 NeuronCores — v2.

Strategy (dst-partitioned edge parallelism, bulk SWDGE gathers):
  - Nodes split into 8 contiguous row-ranges (one per core). Per core,
    dst nodes are packed into NW windows of <=127 nodes.
  - Phase A (per core): project the core's node shard with TensorE:
        [Wh | e_s | e_d] = h_shard @ [Wmat | A_s | A_d] + bias
    Rows [Wh | e_s] (264 bf16, padded to 384 = 768B) go to a DRAM table
    that is AllGathered so every core holds all 50176 rows; e_d rows
    (8 f32, padded to 64 = 256B) stay in a core-local DRAM mini-table.
  - Phase B (per core): edges grouped by destination window; G windows
    form a "group". Per group, three bulk dma_gather calls fetch all
    per-edge data (dma_gather's int16 indices only address 32768 rows,
    so the global table is read through two base offsets — "lo" rows
    [0, 32768) and "hi" rows [32768, NSG) — and every 128-edge chunk is
    packed all-lo or all-hi on the host):
      * table rows by src (one call per lo/hi section)
      * e_d rows by dst from the local mini-table (single call; dst is
        always in the local shard)
    Then per group: w = max(exp(e_s+e_d), exp(0.2(e_s+e_d))) (ACT+DVE),
    one batched DVE op builds all 128-edge one-hot matrices, and per
    window C matmuls accumulate [sum w*Wh | sum w] in PSUM, normalized
    once per window (equivalent to the reference softmax; scores are
    bounded so the max-subtraction is unnecessary).
  - Host does only index/layout work: edge bucketing, packing,
    transposes, and final row/column unscrambles.
"""
import os
import sys

sys.path.insert(0, "/opt/trn_rl_repo")

import numpy as np
import ml_dtypes

import concourse.bass as bass
import concourse.bacc as bacc
import concourse.tile as tile
import concourse.mybir as mybir

BF16 = np.dtype(ml_dtypes.bfloat16)
P = 128
SPLIT = 32768          # dma_gather int16 index limit

FULL_CFG = dict(
    N=50000, F=512, H=8, O=32, ALPHA=0.2, NCORES=8,
    NS=6272, NW=50, G=2, H1=3200,
)

_LAST_RESULTS = {}  # exposed for test.py (exec time etc.)


# --------------------------------------------------------------------------
# Host-side planning
# --------------------------------------------------------------------------

def _plan(cfg, src, dst):
    """Window assignment + lo/hi chunk packing, equalized across cores.

    Returns per-core index arrays (shared shapes) + group metadata
    (identical across cores, baked into the SPMD program).
    """
    import heapq

    N, NCORES = cfg["N"], cfg["NCORES"]
    NS, NW, G = cfg["NS"], cfg["NW"], cfg["G"]
    H1 = cfg["H1"]
    NG = NW // G
    assert NW % G == 0

    deg = np.bincount(dst, minlength=N).astype(np.int64)

    slot_of = np.empty(N, np.int32)   # window within core
    pos_of = np.empty(N, np.int32)    # position within window (0..126)

    for c in range(NCORES):
        lo, hi = NS * c, min(NS * (c + 1), N)
        nodes = np.arange(lo, hi)
        order = nodes[np.argsort(-deg[lo:hi], kind="stable")]
        heap = [(0, 0, w) for w in range(NW)]
        heapq.heapify(heap)
        for n in order:
            load, cnt, w = heapq.heappop(heap)
            slot_of[n] = w
            pos_of[n] = cnt
            cnt += 1
            load += deg[n]
            if cnt < P - 1:  # positions 0..126; 127 reserved for pads
                heapq.heappush(heap, (load, cnt, w))

    # per (core, window, half) edge lists
    core_of = dst // NS
    ed_lists = [[[None, None] for _ in range(NW)] for _ in range(NCORES)]
    for c in range(NCORES):
        m = core_of == c
        s_c, d_c = src[m], dst[m]
        w_c = slot_of[d_c]
        half = ((s_c % NS) >= H1).astype(np.int64)
        key = w_c * 2 + half
        order = np.argsort(key, kind="stable")
        s_c, d_c, key = s_c[order], d_c[order], key[order]
        bounds = np.searchsorted(key, np.arange(2 * NW + 1))
        for w in range(NW):
            for h in (0, 1):
                a, b = bounds[2 * w + h], bounds[2 * w + h + 1]
                ed_lists[c][w][h] = (s_c[a:b], d_c[a:b])

    # chunk counts per (window, half), equalized across cores
    n_chunks = np.zeros((NW, 2), np.int64)
    for c in range(NCORES):
        for w in range(NW):
            for h in (0, 1):
                n = len(ed_lists[c][w][h][0])
                n_chunks[w, h] = max(n_chunks[w, h], -(-n // P))

    # group metadata (identical across cores)
    groups = []
    lo8_off = hi8_off = ed8_off = col_off = 0
    for g in range(NG):
        wins = list(range(g * G, (g + 1) * G))
        K_lo = int(sum(n_chunks[w, 0] for w in wins))
        K_hi = int(sum(n_chunks[w, 1] for w in wins))
        K = K_lo + K_hi
        win_meta = []
        lo_rel = hi_rel = 0
        for w in wins:
            nl, nh = int(n_chunks[w, 0]), int(n_chunks[w, 1])
            win_meta.append(dict(w=w, lo_rel=lo_rel, n_lo=nl,
                                 hi_rel=hi_rel, n_hi=nh))
            lo_rel += nl
            hi_rel += nh
        groups.append(dict(K_lo=K_lo, K_hi=K_hi, K=K,
                           lo8_off=lo8_off, hi8_off=hi8_off,
                           ed8_off=ed8_off, col_off=col_off,
                           wins=win_meta))
        lo8_off += K_lo * 8
        hi8_off += K_hi * 8
        ed8_off += K * 8
        col_off += K
    LO8, HI8, ED8, KT = lo8_off, hi8_off, ed8_off, col_off

    # per-core packed arrays
    idx_lo = np.zeros((NCORES, 128, LO8), np.int16)
    idx_hi = np.zeros((NCORES, 128, HI8), np.int16)
    idx_ed = np.zeros((NCORES, 128, ED8), np.int16)
    dstl = np.full((NCORES, P, KT), 127.0, np.float32)

    for c in range(NCORES):
        ilo = np.zeros(LO8 * 16, np.int64)   # flat idx pos -> table row
        ihi = np.zeros(HI8 * 16, np.int64)
        ied = np.zeros(ED8 * 16, np.int64)
        for gm in groups:
            for h, (ixarr, base8) in enumerate(
                    ((ilo, gm["lo8_off"]), (ihi, gm["hi8_off"]))):
                sec_rel = 0
                for wm in gm["wins"]:
                    s_e, d_e = ed_lists[c][wm["w"]][h]
                    n = len(s_e)
                    i = np.arange(n)
                    pos = base8 * 16 + (sec_rel + i // P) * P + i % P
                    if h == 0:
                        ixarr[pos] = (s_e // NS) * H1 + (s_e % NS)
                    else:
                        ixarr[pos] = ((s_e // NS) * (NS - H1)
                                      + (s_e % NS - H1))
                    # ed idx + dstl live in the unified col space
                    colb = gm["col_off"] + (gm["K_lo"] if h else 0)
                    col = colb + (wm["lo_rel"] if h == 0 else wm["hi_rel"]) \
                        + i // P
                    epos = gm["ed8_off"] * 16 \
                        + ((gm["K_lo"] if h else 0)
                           + (wm["lo_rel"] if h == 0 else wm["hi_rel"])
                           + i // P) * P + i % P
                    ied[epos] = d_e - NS * c
                    dstl[c][i % P, col] = pos_of[d_e]
                    sec_rel += (wm["n_lo"] if h == 0 else wm["n_hi"])
        for flat, arr16 in ((ilo, idx_lo), (ihi, idx_hi), (ied, idx_ed)):
            if flat.size == 0:
                continue
            a = np.zeros((16, flat.size // 16), np.int16)
            a[np.arange(flat.size) % 16, np.arange(flat.size) // 16] = flat
            arr16[c] = np.tile(a, (8, 1))

    return dict(
        groups=groups, LO8=LO8, HI8=HI8, ED8=ED8, KT=KT,
        idx_lo=idx_lo, idx_hi=idx_hi, idx_ed=idx_ed, dstl=dstl,
        slot_of=slot_of, pos_of=pos_of,
    )


def _host_weights(cfg, W, Wb, a, ab):
    """Extended projection weights / bias (o-major, h-inner layout)."""
    H, F, O = W.shape
    a_src, a_dst = a[:, :O], a[:, O:]
    Wmat = W.transpose(1, 2, 0).reshape(F, O * H)          # [F, (o,h)]
    A_s = np.einsum("hfo,ho->fh", W, a_src)
    A_d = np.einsum("hfo,ho->fh", W, a_dst)
    Wext = np.concatenate([Wmat, A_s, A_d], axis=1)        # [F, OH+2H]
    c_s = (Wb * a_src).sum(1)
    c_d = (Wb * a_dst).sum(1)
    bext = np.concatenate([Wb.T.reshape(-1), np.zeros(H, np.float32),
                           c_s + c_d + ab]).astype(np.float32)
    return Wext.astype(np.float32), bext


# --------------------------------------------------------------------------
# Device program
# --------------------------------------------------------------------------

def build_gat_bass(cfg, plan_meta, repeat=1):
    N, F, H, O, NCORES = cfg["N"], cfg["F"], cfg["H"], cfg["O"], cfg["NCORES"]
    NS, NW, G = cfg["NS"], cfg["NW"], cfg["G"]
    HO = H * O
    TDC = HO + H         # useful table row: Wh + e_s = 264
    ROWB = 384           # padded table row elems (768 B)
    EDW = 64             # padded e_d row elems f32 (256 B)
    AD = HO + 2 * H      # phase-A psum width = 272
    NT = NS // P
    KT_F = F // P
    NSG = NS * NCORES

    H1 = cfg["H1"]
    H2 = NS - H1
    T1 = H1 // P            # phase-A tiles in half 1
    NH1, NH2 = H1 * NCORES, H2 * NCORES
    groups = plan_meta["groups"]
    LO8, HI8, ED8, KT = (plan_meta[k] for k in ("LO8", "HI8", "ED8", "KT"))

    bf = mybir.dt.bfloat16
    f32 = mybir.dt.float32
    i16 = mybir.dt.int16

    nc = bacc.Bacc("TRN2", target_bir_lowering=False, debug=False,
                   num_devices=NCORES, num_swdge_queues=4)

    hT = nc.dram_tensor("hT", [F, NS], bf, kind="ExternalInput")
    wext = nc.dram_tensor("wext", [F, AD], bf, kind="ExternalInput")
    bext = nc.dram_tensor("bext", [1, AD], bf, kind="ExternalInput")
    ones1 = nc.dram_tensor("ones1", [1, P], bf, kind="ExternalInput")
    iota = nc.dram_tensor("iota", [P, P], bf, kind="ExternalInput")
    idx_lo = nc.dram_tensor("idx_lo", [128, LO8], i16, kind="ExternalInput")
    idx_hi = nc.dram_tensor("idx_hi", [128, HI8], i16, kind="ExternalInput")
    idx_ed = nc.dram_tensor("idx_ed", [128, ED8], i16, kind="ExternalInput")
    dstl = nc.dram_tensor("dstl", [P, KT], bf, kind="ExternalInput")

    out_local = nc.dram_tensor("out_local", [NW * P, HO], f32,
                               kind="ExternalOutput")

    with tile.TileContext(nc) as tc:
      for _rep in range(repeat):
        with (
            tc.tile_pool(name="dram", bufs=1, space="DRAM") as dram,
            tc.tile_pool(name="const", bufs=1) as cpool,
        ):
            tbl_local = dram.tile([NS, ROWB], bf)
            tbl_g1 = dram.tile([NH1, ROWB], bf, addr_space="Shared")
            tbl_g2 = dram.tile([NH2, ROWB], bf, addr_space="Shared")
            ed_local = dram.tile([NS, EDW], f32)

            iota_t = cpool.tile([P, P], bf)
            nc.sync.dma_start(out=iota_t[:], in_=iota[:, :])
            ones_t = cpool.tile([1, P], bf)
            nc.sync.dma_start(out=ones_t[:], in_=ones1[:, :])
            bext_t = cpool.tile([1, AD], bf)
            nc.sync.dma_start(out=bext_t[:], in_=bext[:, :])
            ilo_t = cpool.tile([128, LO8], i16)
            nc.sync.dma_start(out=ilo_t[:], in_=idx_lo[:, :])
            ihi_t = cpool.tile([128, HI8], i16)
            nc.sync.dma_start(out=ihi_t[:], in_=idx_hi[:, :])
            ied_t = cpool.tile([128, ED8], i16)
            nc.sync.dma_start(out=ied_t[:], in_=idx_ed[:, :])
            dstl_t = cpool.tile([P, KT], bf)
            nc.sync.dma_start(out=dstl_t[:], in_=dstl[:, :])

            # ---------------- Phase A: projection ----------------
            with (
                tc.tile_pool(name="pa_sb", bufs=1) as pa,
                tc.tile_pool(name="pa_ps", bufs=2, space="PSUM") as pa_ps,
            ):
                hT_t = pa.tile([P, KT_F * NS], bf, tag="hT")
                for kk in range(KT_F):
                    nc.sync.dma_start(out=hT_t[:, kk * NS:(kk + 1) * NS],
                                      in_=hT[kk * P:(kk + 1) * P, :])
                wext_t = pa.tile([P, KT_F * AD], bf, tag="wext")
                for kk in range(KT_F):
                    nc.sync.dma_start(out=wext_t[:, kk * AD:(kk + 1) * AD],
                                      in_=wext[kk * P:(kk + 1) * P, :])

                stage = pa.tile([P, NT * TDC], bf, tag="stage")
                ed_stage = pa.tile([P, NT * H], f32, tag="ed_stage")

                for t in range(NT):
                    psA = pa_ps.tile([P, AD], f32, tag="psA")
                    for kk in range(KT_F):
                        nc.tensor.matmul(
                            out=psA[:],
                            lhsT=hT_t[:, kk * NS + t * P: kk * NS + (t + 1) * P],
                            rhs=wext_t[:, kk * AD:(kk + 1) * AD],
                            start=(kk == 0), stop=False)
                    nc.tensor.matmul(out=psA[:], lhsT=ones_t[:],
                                     rhs=bext_t[:], start=False, stop=True)
                    nc.vector.tensor_copy(
                        out=stage[:, t * TDC:(t + 1) * TDC],
                        in_=psA[:, 0:TDC])
                    nc.vector.tensor_copy(
                        out=ed_stage[:, t * H:(t + 1) * H],
                        in_=psA[:, TDC:TDC + H])

                # write only the live 264/8 leading columns of each padded
                # row; DRAM pad columns stay uninitialized (gathered into
                # unused SBUF columns, never read by compute)
                nc.sync.dma_start(
                    out=tbl_local[:].rearrange(
                        "(t p) d -> p t d", p=P)[:, :, 0:TDC],
                    in_=stage[:].rearrange("p (t d) -> p t d", t=NT))
                nc.sync.dma_start(
                    out=ed_local[:].rearrange(
                        "(t p) d -> p t d", p=P)[:, :, 0:H],
                    in_=ed_stage[:].rearrange("p (t d) -> p t d", t=NT))

            if cfg.get("skip_collective"):
                nc.sync.dma_start(out=tbl_g1[0:H1, :],
                                  in_=tbl_local[0:H1, :])
                nc.sync.dma_start(out=tbl_g2[0:H2, :],
                                  in_=tbl_local[H1:NS, :])
            else:
                # two half-shard AllGathers: the first half-table is ready
                # while the second collective still runs, so the first wave
                # of table gathers overlaps it
                nc.gpsimd.collective_compute(
                    "AllGather",
                    mybir.AluOpType.bypass,
                    replica_groups=[list(range(NCORES))],
                    ins=[tbl_local[0:H1, :]],
                    outs=[tbl_g1.opt()],
                )
                nc.gpsimd.collective_compute(
                    "AllGather",
                    mybir.AluOpType.bypass,
                    replica_groups=[list(range(NCORES))],
                    ins=[tbl_local[H1:NS, :]],
                    outs=[tbl_g2.opt()],
                )

            # ---------------- Phase B: edges ----------------
            with (
                tc.tile_pool(name="g_sb", bufs=3) as gp,
                tc.tile_pool(name="ed_sb", bufs=3) as edp,
                tc.tile_pool(name="w_sb", bufs=3) as wpool,
                tc.tile_pool(name="a_sb", bufs=2) as apool,
                tc.tile_pool(name="o_sb", bufs=3) as opool,
                tc.tile_pool(name="agg_ps", bufs=6, space="PSUM") as aggp,
            ):
                # HW limit: a dma_gather's descriptors must fit the SWDGE
                # ring -> cap num_idxs per call (empirically 512 ok, 1920
                # wedges the device). Rotate the 4 SWDGE queues so the next
                # call's descriptor generation overlaps in-flight transfers.
                CAP = int(os.environ.get("GAT_CAP", "7"))
                _q = [0]

                def capped_gather(dst_tile, col0, ncols, in_ap, idx_t,
                                  idx8_off, elem):
                    c, off, left = col0, idx8_off, ncols
                    while left > 0:
                        n = min(left, CAP)
                        nc.gpsimd.dma_gather(
                            dst_tile[:, c * elem:(c + n) * elem].rearrange(
                                "p (k d) -> p k d", k=n),
                            in_ap, idx_t[:, off:off + n * 8],
                            n * P, n * P, elem,
                            queue_num=_q[0])
                        _q[0] = (_q[0] + 1) % 4
                        c += n
                        off += n * 8
                        left -= n

                for gm in groups:
                    K, KL, KH = gm["K"], gm["K_lo"], gm["K_hi"]
                    # ed gather first: it only depends on phase A's local
                    # write, so it can proceed while the AllGather (which
                    # gates the table gathers) is still in flight.
                    ed_t = edp.tile([P, K * EDW], f32, tag="ed")
                    capped_gather(ed_t, 0, K, ed_local[:, :],
                                  ied_t, gm["ed8_off"], EDW)
                    g_t = gp.tile([P, K * ROWB], bf, tag="g")
                    if KL:
                        capped_gather(g_t, 0, KL, tbl_g1[:, :],
                                      ilo_t, gm["lo8_off"], ROWB)
                    if KH:
                        capped_gather(g_t, KL, KH, tbl_g2[:, :],
                                      ihi_t, gm["hi8_off"], ROWB)

                    g4 = g_t[:].rearrange("p (k d) -> p k d", k=K)
                    ed4 = ed_t[:].rearrange("p (k d) -> p k d", k=K)

                    # s = e_s + e_d
                    s_t = wpool.tile([P, K * H], f32, tag="s")
                    nc.vector.tensor_tensor(
                        out=s_t[:].rearrange("p (k x) -> p k x", k=K),
                        in0=g4[:, :, HO:TDC],
                        in1=ed4[:, :, 0:H],
                        op=mybir.AluOpType.add)

                    # w = max(exp(s), exp(0.2 s)) -> g cols HO:TDC (bf16)
                    w1 = wpool.tile([P, K * H], f32, tag="w1")
                    nc.scalar.activation(out=w1[:], in_=s_t[:],
                                         func=mybir.ActivationFunctionType.Exp)
                    w2 = wpool.tile([P, K * H], f32, tag="w2")
                    nc.scalar.activation(out=w2[:], in_=s_t[:],
                                         func=mybir.ActivationFunctionType.Exp,
                                         scale=float(cfg["ALPHA"]))
                    nc.vector.tensor_tensor(
                        out=g4[:, :, HO:TDC],
                        in0=w1[:].rearrange("p (k x) -> p k x", k=K),
                        in1=w2[:].rearrange("p (k x) -> p k x", k=K),
                        op=mybir.AluOpType.max)

                    # Wh *= w (broadcast over O), in place
                    nc.vector.tensor_tensor(
                        out=g4[:, :, 0:HO].rearrange(
                            "p k (o x) -> p k o x", o=O),
                        in0=g4[:, :, 0:HO].rearrange(
                            "p k (o x) -> p k o x", o=O),
                        in1=g4[:, :, HO:TDC][:, :, None, :].to_broadcast(
                            [P, K, O, H]),
                        op=mybir.AluOpType.mult)

                    # batched one-hot: a[p, k, v] = (iota[v] == dstl[p, k])
                    a_t = apool.tile([P, K * P], bf, tag="a")
                    nc.vector.tensor_tensor(
                        out=a_t[:].rearrange("p (k v) -> p k v", k=K),
                        in0=iota_t[:][:, None, :].to_broadcast([P, K, P]),
                        in1=dstl_t[:, gm["col_off"]:gm["col_off"] + K][
                            :, :, None].to_broadcast([P, K, P]),
                        op=mybir.AluOpType.is_equal)

                    for wm in gm["wins"]:
                        cols = (list(range(wm["lo_rel"],
                                           wm["lo_rel"] + wm["n_lo"]))
                                + list(range(KL + wm["hi_rel"],
                                             KL + wm["hi_rel"] + wm["n_hi"])))
                        o_t = opool.tile([P, HO], f32, tag="o")
                        if not cols:
                            nc.vector.memset(o_t[:], 0.0)
                        else:
                            agg = aggp.tile([P, TDC], f32, tag="agg")
                            for j, col in enumerate(cols):
                                nc.tensor.matmul(
                                    out=agg[:],
                                    lhsT=a_t[:, col * P:(col + 1) * P],
                                    rhs=g_t[:, col * ROWB:col * ROWB + TDC],
                                    start=(j == 0), stop=(j == len(cols) - 1))
                            den = opool.tile([P, H], f32, tag="den")
                            nc.vector.tensor_scalar(
                                out=den[:], in0=agg[:, HO:TDC],
                                scalar1=1e-30, scalar2=None,
                                op0=mybir.AluOpType.max)
                            rec = opool.tile([P, H], f32, tag="rec")
                            nc.vector.reciprocal(out=rec[:], in_=den[:])
                            nc.vector.tensor_tensor(
                                out=o_t[:].rearrange("p (o x) -> p o x", o=O),
                                in0=agg[:, 0:HO].rearrange(
                                    "p (o x) -> p o x", o=O),
                                in1=rec[:][:, None, :].to_broadcast([P, O, H]),
                                op=mybir.AluOpType.mult)
                        nc.sync.dma_start(
                            out=out_local[wm["w"] * P:(wm["w"] + 1) * P, :],
                            in_=o_t[:])

    return nc


# --------------------------------------------------------------------------
# Execution (PJRT via axon; chained-execution slope timing)
# --------------------------------------------------------------------------

def _run_pjrt_timed(nc, in_maps, n_cores, n_reps=1, chain=None):
    import time

    import jax
    from jax.sharding import Mesh, PartitionSpec
    from jax.experimental.shard_map import shard_map

    from concourse import bass2jax
    from concourse import mybir as mb

    bass2jax.install_neuronx_cc_hook()

    partition_name = (nc.partition_id_tensor.name
                      if nc.partition_id_tensor else None)

    in_names, out_names, out_avals, zero_outs = [], [], [], []
    for alloc in nc.m.functions[0].allocations:
        if not isinstance(alloc, mb.MemoryLocationSet):
            continue
        name = alloc.memorylocations[0].name
        if alloc.kind == "ExternalInput":
            if name != partition_name:
                in_names.append(name)
        elif alloc.kind == "ExternalOutput":
            shape = tuple(alloc.tensor_shape)
            dtype = mb.dt.np(alloc.dtype)
            out_names.append(name)
            out_avals.append(jax.core.ShapedArray(shape, dtype))
            zero_outs.append(np.zeros(shape, dtype))
    n_params = len(in_names)
    n_outs = len(out_avals)
    all_in_names = list(in_names) + out_names
    if partition_name is not None:
        all_in_names.append(partition_name)
    donate = tuple(range(n_params, n_params + n_outs))

    def _body(*args):
        operands = list(args)
        if partition_name is not None:
            operands.append(bass2jax.partition_id_tensor())
        outs = bass2jax._bass_exec_p.bind(
            *operands,
            out_avals=tuple(out_avals),
            in_names=tuple(all_in_names),
            out_names=tuple(out_names),
            lowering_input_output_aliases=(),
            sim_require_finite=True,
            sim_require_nnan=True,
            nc=nc,
        )
        return tuple(outs)

    devices = jax.devices()[:n_cores]
    mesh = Mesh(np.asarray(devices), ("core",))
    in_specs = (PartitionSpec("core"),) * (n_params + n_outs)
    out_specs = (PartitionSpec("core"),) * len(out_names)
    sharded = jax.jit(
        shard_map(_body, mesh=mesh, in_specs=in_specs, out_specs=out_specs,
                  check_rep=False),
        donate_argnums=donate, keep_unused=True)

    sharding = jax.sharding.NamedSharding(mesh, PartitionSpec("core"))
    concat_in = [
        jax.device_put(
            np.concatenate([np.asarray(in_maps[c][name])
                            for c in range(n_cores)], axis=0), sharding)
        for name in in_names
    ]

    def fresh_zeros():
        return [
            jax.device_put(
                np.zeros((n_cores * z.shape[0], *z.shape[1:]), z.dtype),
                sharding)
            for z in zero_outs
        ]

    def run_chain(k):
        outs = fresh_zeros()
        for o in outs:
            o.block_until_ready()
        t0 = time.perf_counter()
        for _ in range(k):
            outs = sharded(*concat_in, *outs)
        for o in outs:
            o.block_until_ready()
        return time.perf_counter() - t0, outs

    # warmup / compile
    _, out_arrs = run_chain(1)

    if chain:
        # Per-execution device time via chained-run slope. Chained calls
        # amortize the fixed dispatch latency; using the min totals at two
        # chain lengths rejects scheduler noise (fixed latency cancels).
        k1, k2 = chain
        t1s, t2s = [], []
        for _ in range(max(1, n_reps)):
            t1, _ = run_chain(k1)
            t2, out_arrs = run_chain(k2)
            t1s.append(t1)
            t2s.append(t2)
        slope = (min(t2s) - min(t1s)) / (k2 - k1)
        _LAST_RESULTS["wall_times_s"] = t1s + t2s
        _LAST_RESULTS["slopes_s"] = [
            (b - a) / (k2 - k1) for a, b in zip(t1s, t2s)]
        _LAST_RESULTS["exec_time_ns"] = int(slope * 1e9)
    else:
        times = []
        for _ in range(max(1, n_reps)):
            t, out_arrs = run_chain(1)
            times.append(t)
        _LAST_RESULTS["wall_times_s"] = times
        _LAST_RESULTS["exec_time_ns"] = int(min(times) * 1e9)

    return [
        {name: np.asarray(out_arrs[i]).reshape(n_cores, *out_avals[i].shape)[c]
         for i, name in enumerate(out_names)}
        for c in range(n_cores)
    ]


def _chain_totals(nc, in_maps, n_cores, k, n_reps):
    """Wall totals of `n_reps` chains of `k` dispatches of nc's program."""
    import time

    import jax
    from jax.sharding import Mesh, PartitionSpec
    from jax.experimental.shard_map import shard_map

    from concourse import bass2jax
    from concourse import mybir as mb

    bass2jax.install_neuronx_cc_hook()
    partition_name = (nc.partition_id_tensor.name
                      if nc.partition_id_tensor else None)
    in_names, out_names, out_avals, zero_outs = [], [], [], []
    for alloc in nc.m.functions[0].allocations:
        if not isinstance(alloc, mb.MemoryLocationSet):
            continue
        name = alloc.memorylocations[0].name
        if alloc.kind == "ExternalInput":
            if name != partition_name:
                in_names.append(name)
        elif alloc.kind == "ExternalOutput":
            shape = tuple(alloc.tensor_shape)
            dtype = mb.dt.np(alloc.dtype)
            out_names.append(name)
            out_avals.append(jax.core.ShapedArray(shape, dtype))
            zero_outs.append(np.zeros(shape, dtype))
    n_params = len(in_names)
    all_in_names = list(in_names) + out_names
    if partition_name is not None:
        all_in_names.append(partition_name)
    donate = tuple(range(n_params, n_params + len(out_avals)))

    def _body(*args):
        operands = list(args)
        if partition_name is not None:
            operands.append(bass2jax.partition_id_tensor())
        return tuple(bass2jax._bass_exec_p.bind(
            *operands, out_avals=tuple(out_avals),
            in_names=tuple(all_in_names), out_names=tuple(out_names),
            lowering_input_output_aliases=(),
            sim_require_finite=True, sim_require_nnan=True, nc=nc))

    devices = jax.devices()[:n_cores]
    mesh = Mesh(np.asarray(devices), ("core",))
    nio = n_params + len(out_avals)
    sharded = jax.jit(
        shard_map(_body, mesh=mesh, in_specs=(PartitionSpec("core"),) * nio,
                  out_specs=(PartitionSpec("core"),) * len(out_names),
                  check_rep=False),
        donate_argnums=donate, keep_unused=True)
    sharding = jax.sharding.NamedSharding(mesh, PartitionSpec("core"))
    concat_in = [
        jax.device_put(
            np.concatenate([np.asarray(in_maps[c][name])
                            for c in range(n_cores)], axis=0), sharding)
        for name in in_names
    ]

    def run_chain(kk):
        outs = [jax.device_put(
            np.zeros((n_cores * z.shape[0], *z.shape[1:]), z.dtype), sharding)
            for z in zero_outs]
        for o in outs:
            o.block_until_ready()
        t0 = time.perf_counter()
        for _ in range(kk):
            outs = sharded(*concat_in, *outs)
        for o in outs:
            o.block_until_ready()
        return time.perf_counter() - t0

    run_chain(1)  # warmup/compile
    return [run_chain(k) for _ in range(n_reps)]


# --------------------------------------------------------------------------
# Host entry point
# --------------------------------------------------------------------------

def _run(cfg, h, src, dst, W, Wb, a, ab, use_sim=False, timing=False):
    N, F, H, O, NCORES = cfg["N"], cfg["F"], cfg["H"], cfg["O"], cfg["NCORES"]
    NS, NW = cfg["NS"], cfg["NW"]
    HO = H * O

    h = np.asarray(h, np.float32)
    src = np.asarray(src).astype(np.int64)
    dst = np.asarray(dst).astype(np.int64)
    W = np.asarray(W, np.float32)
    Wb = np.asarray(Wb, np.float32)
    a = np.asarray(a, np.float32)
    ab = np.asarray(ab, np.float32)

    plan = _plan(cfg, src, dst)
    Wext, bext = _host_weights(cfg, W, Wb, a, ab)

    NSG = NS * NCORES
    h_pad = np.zeros((NSG, F), np.float32)
    h_pad[:N] = h
    iota_np = np.broadcast_to(np.arange(P, dtype=np.float32),
                              (P, P)).astype(BF16)
    in_maps = []
    for c in range(NCORES):
        in_maps.append({
            "hT": np.ascontiguousarray(
                h_pad[NS * c:NS * (c + 1)].T).astype(BF16),
            "wext": Wext.astype(BF16),
            "bext": bext.reshape(1, -1).astype(BF16),
            "ones1": np.ones((1, P), BF16),
            "iota": iota_np,
            "idx_lo": plan["idx_lo"][c],
            "idx_hi": plan["idx_hi"][c],
            "idx_ed": plan["idx_ed"][c],
            "dstl": plan["dstl"][c].astype(BF16),
        })

    nc = build_gat_bass(cfg, plan)
    nc.compile()

    if use_sim:
        from concourse import bass_interp
        sim = bass_interp.MultiCoreSim(nc, NCORES)
        for c in range(NCORES):
            for k, v in in_maps[c].items():
                sim.cores[c].tensor(k)[:] = v
        sim.simulate()
        outs = [np.array(sim.cores[c].mem_tensor("out_local"))
                for c in range(NCORES)]
    else:
        results = _run_pjrt_timed(nc, in_maps, NCORES, n_reps=1)
        outs = [results[c]["out_local"] for c in range(NCORES)]
        if timing:
            # Device-time measurement: the per-dispatch overhead of this
            # PJRT/axon path (~1.5 ms, measured with a trivial kernel) dwarfs
            # the kernel, so time a variant program whose body runs REP times
            # per dispatch and take the slope over REP via chained runs at
            # two chain lengths (min over reps cancels dispatch noise).
            REP, K, NR = cfg.get("REP", 5), 24, 3
            ncR = build_gat_bass(cfg, plan, repeat=REP)
            ncR.compile()
            t1s = _chain_totals(nc, in_maps, NCORES, K, NR)
            t2s = _chain_totals(ncR, in_maps, NCORES, K, NR)
            d = (min(t2s) - min(t1s)) / (K * (REP - 1))
            _LAST_RESULTS["wall_times_s"] = t1s + t2s
            _LAST_RESULTS["exec_time_ns"] = int(d * 1e9)

    # unscramble rows + columns
    slot_of, pos_of = plan["slot_of"], plan["pos_of"]
    nodes = np.arange(N)
    rows = slot_of[nodes] * P + pos_of[nodes]
    out = np.empty((N, HO), np.float32)
    for c in range(NCORES):
        lo, hi = NS * c, min(NS * (c + 1), N)
        out[lo:hi] = outs[c][rows[lo:hi]]
    # column map: ref col h*O+o <- ours o*H+h
    hh, oo = np.meshgrid(np.arange(H), np.arange(O), indexing="ij")
    colmap = (oo * H + hh).reshape(-1)
    return out[:, colmap]


def kernel(h, src, dst, W, Wb, a, ab):
    cfg = dict(FULL_CFG)
    timing = os.environ.get("GAT_TRACE", "0") == "1"
    use_sim = os.environ.get("GAT_SIM", "0") == "1"
    return _run(cfg, h, src, dst, W, Wb, a, ab, use_sim=use_sim,
                timing=timing)
